# revision 10
# baseline (speedup 1.0000x reference)
"""Multi-head cross-attention Trainium2 Bass kernel, SPMD over 8 NeuronCores.

Sharding: core c handles batch b = c//2 and head group g = c%2 (8 of 16 heads).
Each core computes a partial output projection (its heads' W_o rows); the host
sums the two partials per batch element.

Device pipeline per core (all matmuls bf16 with fp32 PSUM accumulation):
  kT = (Wk^T x^T)          [512 hd, 2048 kseq]   (per-partition bias b_k)
  v  = (x Wv)              [2048 kseq, 8*64]
  qT = (Wq^T y^T)          [512 hd, 1024 q]      (per-partition bias b_q)
  per (head-pair, q-tile, k-chunk):
      S^T[k, q|q'] = kT_h^T-chunk @ qT_h for both heads of the pair
        (K=64 row-tiled at partitions 0/64 -> concurrent in the PE array)
      em = exp(0.125 * S^T)  (one ACT op per pair)
      em *= maskT            (one DVE mul per pair, head-broadcast)
      attnV: col-tiled pair into ONE psum bank: h0 -> vacc[0:64],
        h1 -> vacc[64:128]  (tile_position (0,0)/(0,64), concurrent)
      denominators: every 2nd kc, a 4-way col-tiled batch of ones-matmuls
        (strips at psum partitions 0/32/64/96) accumulates sum_k em for
        (even-kc h0, even h1, odd h0, odd h1) -> 2 kc amortized per 512 cyc
  normalize per block: dsum rows -> reciprocal_approx_fast -> bf16 ->
      one K=2 selector matmul (E2) broadcasts [2,512] over the 128
      partitions -> one [128,512] DVE mul into vals
  out_partial = vals^T-chunks @ Wo-rows  -> [1024 q, 1024 D]
Load phase: inputs are host-preswizzled to SBUF layout so each tensor is one
large contiguous DMA (the per-DMA issue cost on the Sync queue is ~0.6us;
many small DMAs made the old load phase issue-bound).  ~30 junk matmuls on a
zeroed tile warm the PE clock (HAM) during the DMA window.
b_v and b_o fold into a host-side constant row (attn rows sum to 1).
"""

import sys
from collections import deque

import numpy as np
import ml_dtypes

if "/opt/trn_rl_repo" not in sys.path:
    sys.path.insert(0, "/opt/trn_rl_repo")

BF = ml_dtypes.bfloat16

B, NKV, NQ, D, H = 4, 2048, 1024, 1024, 16
HD = D // H          # 64
NHL = 8              # heads per core (local)
P = 128
DC = D // P          # 8 contraction chunks over model dim
KC = NKV // P        # 16 key-seq chunks
QT = NQ // 512       # 2 q tiles of 512 for attention
MT = 4               # hd-dim chunks of kT/qT (512/128)

_CACHE = {}


def _build_program():
    import concourse.bass as bass
    import concourse.mybir as mybir
    import concourse.tile as tile
    from concourse import bacc

    f32 = mybir.dt.float32
    bf16 = mybir.dt.bfloat16

    nc = bacc.Bacc(
        "TRN2", target_bir_lowering=False, debug=False, num_devices=8
    )

    # host-preswizzled inputs: each is one contiguous DMA into its SBUF tile
    xT_d = nc.dram_tensor("xT", [512, 4096], bf16, kind="ExternalInput").ap()
    yT_d = nc.dram_tensor("yT", [P, DC * NQ], bf16, kind="ExternalInput").ap()
    maskT_d = nc.dram_tensor(
        "maskT", [P, KC * NQ], bf16, kind="ExternalInput"
    ).ap()
    wk_d = nc.dram_tensor("wk", [P, DC * 512], bf16, kind="ExternalInput").ap()
    wv_d = nc.dram_tensor("wv", [P, DC * 512], bf16, kind="ExternalInput").ap()
    wq_d = nc.dram_tensor("wq", [P, DC * 512], bf16, kind="ExternalInput").ap()
    wo_d = nc.dram_tensor("wo", [P, MT * D], bf16, kind="ExternalInput").ap()
    bk_d = nc.dram_tensor("bk", [P, MT], f32, kind="ExternalInput").ap()
    bq_d = nc.dram_tensor("bq", [P, MT], f32, kind="ExternalInput").ap()
    out_d = nc.dram_tensor("out", [NQ, D], bf16, kind="ExternalOutput").ap()

    Exp = mybir.ActivationFunctionType.Exp

    with tile.TileContext(nc) as tc:
        with (
            tc.tile_pool(name="persist", bufs=1) as persist,
            tc.tile_pool(name="work", bufs=3) as work,
            tc.tile_pool(name="empool", bufs=4) as empool,
            tc.tile_pool(name="pmm", bufs=1, space="PSUM") as pmm,
            tc.tile_pool(name="pvac", bufs=2, space="PSUM") as pvac,
            tc.tile_pool(name="pdac", bufs=1, space="PSUM") as pdac,
            tc.tile_pool(name="psc", bufs=2, space="PSUM") as psc,
        ):
            # ---- persistent SBUF tiles ----
            wk_sb = persist.tile([P, DC * 512], bf16, tag="wk", name="wk")
            wv_sb = persist.tile([P, DC * 512], bf16, tag="wv", name="wv")
            wq_sb = persist.tile([P, DC * 512], bf16, tag="wq", name="wq")
            wo_sb = persist.tile([P, MT * D], bf16, tag="wo", name="wo")
            xT_sb = persist.tile([P, DC * NKV], bf16, tag="xT", name="xT")
            yT_sb = persist.tile([P, DC * NQ], bf16, tag="yT", name="yT")
            maskT_sb = persist.tile(
                [P, KC * NQ], bf16, tag="mT", name="mT"
            )
            bk_sb = persist.tile([P, MT], f32, tag="bk", name="bk")
            bq_sb = persist.tile([P, MT], f32, tag="bq", name="bq")

            def xs(d, lo, hi):     # xT chunk d, key-cols lo:hi
                return xT_sb[:, d * NKV + lo:d * NKV + hi]

            def ys(d, lo, hi):
                return yT_sb[:, d * NQ + lo:d * NQ + hi]

            # warm tile + constants
            warm = persist.tile([P, 256], bf16, tag="warm", name="warm")
            nc.gpsimd.memset(warm, 0.0)
            ones_col = persist.tile([P, 1], bf16, tag="onesc", name="onesc")
            nc.gpsimd.memset(ones_col, 1.0)
            # selector for the denominator broadcast: row 0 -> parts 0-63,
            # row 32 -> parts 64-127 (K=33 matmul; rows 1-31 are zero)
            e33_sb = persist.tile([33, P], bf16, tag="e33", name="e33")
            nc.gpsimd.memset(e33_sb, 0.0)
            nc.gpsimd.memset(e33_sb[0:1, 0:HD], 1.0)
            nc.gpsimd.memset(e33_sb[32:33, HD:2 * HD], 1.0)

            # ---- HAM warmup: junk matmuls on the zeroed tile ----
            wps = psc.tile([P, 1024], f32, tag="sc", name="warmps")
            for i in range(30):
                nc.tensor.matmul(
                    wps[:, 0:256], lhsT=warm[:, 0:P], rhs=warm,
                    start=True, stop=True,
                )

            # ---- input DMAs: one per tensor (xT by column-quarters) ----
            nc.sync.dma_start(wk_sb, wk_d)
            nc.sync.dma_start(bk_sb, bk_d)
            nc.sync.dma_start(bq_sb, bq_d)
            nc.sync.dma_start(wv_sb, wv_d)
            x3 = xT_sb.rearrange("p (d c) -> p d c", d=DC)
            for qb in range(4):
                nc.sync.dma_start(
                    x3[:, :, qb * 512:(qb + 1) * 512],
                    xT_d[qb * P:(qb + 1) * P, :].rearrange(
                        "p (d c) -> p d c", d=DC),
                )
            nc.sync.dma_start(wq_sb, wq_d)
            nc.sync.dma_start(yT_sb, yT_d)
            for mq in range(4):
                nc.sync.dma_start(
                    maskT_sb[:, mq * 4096:(mq + 1) * 4096],
                    maskT_d[:, mq * 4096:(mq + 1) * 4096],
                )
            nc.sync.dma_start(wo_sb, wo_d)

            kT_sb = [
                persist.tile([P, NKV], bf16, tag=f"kT{m}", name=f"kT{m}")
                for m in range(MT)
            ]
            qT_sb = [
                persist.tile([P, NQ], bf16, tag=f"qT{m}", name=f"qT{m}")
                for m in range(MT)
            ]
            v_sb = [
                persist.tile([P, NHL * HD], bf16, tag=f"v{i}", name=f"v{i}")
                for i in range(KC)
            ]
            vals_sb = [
                persist.tile([P, NQ], bf16, tag=f"vals{c}", name=f"vals{c}")
                for c in range(MT)
            ]

            # ---- projection op queues (closures; chain tail adds bias) ----
            def proj_ops(m, which, ns=None, pool=None):
                w_sb, dst, bias, ncols = (
                    (wk_sb, kT_sb, bk_sb, NKV) if which == "k"
                    else (wq_sb, qT_sb, bq_sb, NQ)
                )
                src = xs if which == "k" else ys
                pool_ = pool if pool is not None else pmm
                tag = "sc" if pool_ is psc else "mm"
                ops = []
                hold = {}
                nlist = list(range(ncols // 512) if ns is None else ns)
                for n in nlist:
                    for d in range(DC):
                        def op(m=m, n=n, d=d, w_sb=w_sb, dst=dst, bias=bias,
                               which=which, src=src, pool_=pool_, tag=tag):
                            if d == 0:
                                hold[n] = pool_.tile(
                                    [P, 512] if tag == "mm" else [P, 1024],
                                    f32, tag=tag, name=f"pj{which}{m}_{n}"
                                )
                            ps = hold[n][:, 0:512]
                            nc.tensor.matmul(
                                ps,
                                lhsT=w_sb[:, d * 512 + m * P:
                                          d * 512 + (m + 1) * P],
                                rhs=src(d, n * 512, (n + 1) * 512),
                                start=(d == 0),
                                stop=(d == DC - 1),
                            )
                            if d == DC - 1:
                                nc.vector.tensor_scalar_add(
                                    dst[m][:, n * 512:(n + 1) * 512],
                                    ps, bias[:, m:m + 1]
                                )
                        ops.append(op)
                return ops

            def v_ops(i, pool_):
                tag = "sc" if pool_ is psc else "mm"
                ops = []
                hold = {}

                def mk(d, i=i, pool_=pool_, tag=tag):
                    def op(d=d):
                        if d == 0:
                            hold[0] = pool_.tile(
                                [P, 512] if tag == "mm" else [P, 1024],
                                f32, tag=tag, name=f"ps_v{i}"
                            )
                        nc.tensor.matmul(
                            hold[0][:, 0:512],
                            lhsT=xs(d, i * P, (i + 1) * P),
                            rhs=wv_sb[:, d * 512:(d + 1) * 512],
                            start=(d == 0),
                            stop=(d == DC - 1),
                        )
                        if d == DC - 1:
                            nc.vector.tensor_copy(v_sb[i], hold[0][:, 0:512])
                    return op
                for d in range(DC):
                    ops.append(mk(d))
                return ops

            # upfront: kT chunk-0 chains as their xT quarters land, early v
            # chains, then qT chunk-0 — attention starts ASAP so ACT/DVE ramp
            for op in proj_ops(0, "k", ns=[0], pool=psc):
                op()
            for i in (0, 1, 2, 3):
                for op in v_ops(i, psc):
                    op()
            for op in proj_ops(0, "k", ns=[1], pool=psc):
                op()
            for i in (4, 5, 6, 7):
                for op in v_ops(i, psc):
                    op()
            for op in proj_ops(0, "k", ns=[2], pool=psc):
                op()
            for i in (8, 9):
                for op in v_ops(i, psc):
                    op()
            for op in proj_ops(0, "k", ns=[3], pool=psc):
                op()
            for i in (10, 11):
                for op in v_ops(i, psc):
                    op()
            for op in proj_ops(0, "q", pool=psc):
                op()

            # deferred work, drained inside the attention loops:
            # hp0 gets the v tail + chunk-1 projections, etc.
            pending = {
                0: deque(
                    v_ops(12, pmm) + v_ops(13, pmm) + v_ops(14, pmm)
                    + v_ops(15, pmm)
                    + proj_ops(1, "k") + proj_ops(1, "q")
                ),
                1: deque(proj_ops(2, "k") + proj_ops(2, "q")),
                2: deque(proj_ops(3, "k") + proj_ops(3, "q")),
                3: deque(),
            }

            # ---- output-projection chains (closures) ----
            def wo_ops(t2s, pool_pick):
                ops = []
                hold = {}
                for t2 in t2s:
                    for n in range(D // 512):
                        for c in range(MT):
                            def op(t2=t2, n=n, c=c):
                                if c == 0:
                                    pool, tag = pool_pick(t2, n)
                                    hold[(t2, n)] = pool.tile(
                                        [P, 512] if tag == "mm"
                                        else [P, 1024],
                                        f32, tag=tag, name=f"ps_o{t2}_{n}"
                                    )
                                ps_o = hold[(t2, n)][:, 0:512]
                                nc.tensor.matmul(
                                    ps_o,
                                    lhsT=vals_sb[c][:, t2 * P:(t2 + 1) * P],
                                    rhs=wo_sb[:, c * D + n * 512:
                                              c * D + (n + 1) * 512],
                                    start=(c == 0),
                                    stop=(c == MT - 1),
                                )
                                if c == MT - 1:
                                    ot = work.tile(
                                        [P, 512], bf16, tag="ot",
                                        name=f"ot{t2}_{n}", bufs=3
                                    )
                                    nc.scalar.copy(ot, ps_o)
                                    nc.sync.dma_start(
                                        out_d[t2 * P:(t2 + 1) * P,
                                              n * 512:(n + 1) * 512], ot
                                    )
                            ops.append(op)
                return ops

            wo_first = deque(wo_ops(range(0, 4), lambda t2, n: (pmm, "mm")))

            # ---- attention ----
            norm_a = deque()
            norm_b = deque()

            def make_norm(hp, t, ut, ds):
                qs = slice(t * 512, (t + 1) * 512)

                def na():
                    rec = work.tile([33, 512], f32, tag="rec",
                                    name=f"rec{hp}_{t}", bufs=2)
                    nc.vector.reciprocal_approx_fast(rec, ds)
                    rb = work.tile([33, 512], bf16, tag="rb",
                                   name=f"rb{hp}_{t}", bufs=2)
                    nc.vector.tensor_copy(rb, rec)
                    na.rb = rb

                def nb():
                    bps = pmm.tile([P, 512], f32, tag="mm",
                                   name=f"bps{hp}_{t}")
                    nc.tensor.matmul(
                        bps, lhsT=e33_sb, rhs=na.rb, start=True, stop=True
                    )
                    nc.vector.tensor_mul(vals_sb[hp][:, qs], ut, bps)
                return na, nb

            for hp in range(MT):
                h0, h1 = 2 * hp, 2 * hp + 1
                q = pending[hp]
                for t in range(QT):
                    qs = slice(t * 512, (t + 1) * 512)
                    slots_left = (QT - t) * KC
                    vacc = pvac.tile([P, 512], f32, tag="vacc",
                                     name=f"vacc{hp}_{t}")
                    dacc = pdac.tile([P, 512], f32, tag="dacc",
                                     name=f"dacc{hp}_{t}")
                    prev_em = None
                    for kc in range(KC):
                        n_emit = -(-len(q) // slots_left)  # ceil
                        for _ in range(min(n_emit, len(q))):
                            q.popleft()()
                        slots_left -= 1
                        if kc == 2 and norm_a:
                            norm_a.popleft()()
                        if kc == 6 and norm_b:
                            norm_b.popleft()()
                        if hp == 3 and t == 1 and kc >= 8:
                            for _ in range(4):
                                if wo_first:
                                    wo_first.popleft()()

                        sp2 = psc.tile(
                            [P, 1024], f32, tag="sc", name=f"sp{hp}_{t}_{kc}"
                        )
                        for a in range(2):
                            po = a * HD
                            nc.tensor.matmul(
                                sp2[:, a * 512:(a + 1) * 512],
                                lhsT=kT_sb[hp][po:po + HD, kc * P:(kc + 1) * P],
                                rhs=qT_sb[hp][po:po + HD, qs],
                                start=True,
                                stop=True,
                            )
                        em2 = empool.tile(
                            [P, 1024], bf16, tag="em", name=f"em{hp}_{t}_{kc}"
                        )
                        nc.scalar.activation(em2, sp2, Exp, scale=0.125)
                        mb = (maskT_sb[:, kc * NQ + t * 512:
                                       kc * NQ + (t + 1) * 512]
                              .rearrange("p (o q) -> p o q", o=1)
                              .broadcast_to([P, 2, 512]))
                        em3 = em2.rearrange("p (o q) -> p o q", o=2)
                        nc.vector.tensor_mul(em3, em3, mb)
                        # attnV: col-tiled pair into one bank (disjoint
                        # partition ranges -> independent accum groups)
                        for a, h in enumerate((h0, h1)):
                            nc.tensor.matmul(
                                vacc[a * HD:(a + 1) * HD, :],
                                lhsT=v_sb[kc][:, h * HD:(h + 1) * HD],
                                rhs=em2[:, a * 512:(a + 1) * 512],
                                start=(kc == 0),
                                stop=(kc == KC - 1),
                            )
                        # denominators: 4-way col-tiled ones-matmuls,
                        # strips (0,32,64,96) <- (even h0, even h1, odd h0,
                        # odd h1); accumulated over the 8 batches
                        if kc % 2 == 1:
                            for j, (emt, a) in enumerate(
                                ((prev_em, 0), (prev_em, 1),
                                 (em2, 0), (em2, 1))
                            ):
                                nc.tensor.matmul(
                                    dacc[32 * j:32 * j + 1, :],
                                    lhsT=ones_col,
                                    rhs=emt[:, a * 512:(a + 1) * 512],
                                    start=(kc == 1),
                                    stop=(kc == KC - 1),
                                    tile_position=(0, 32 * j),
                                )
                        prev_em = em2
                    # block epilogue: free vacc/dacc fast, defer the rest.
                    # ds rows 0/32 = h0/h1 denominators (odd-kc strips at
                    # parts 64/96 copied down, added to even strips 0/32;
                    # rows 1-31 are junk and never read downstream)
                    ut = work.tile([P, 512], f32, tag="ut",
                                   name=f"ut{hp}_{t}", bufs=2)
                    nc.vector.tensor_copy(ut, vacc)
                    thi = work.tile([33, 512], f32, tag="thi",
                                    name=f"thi{hp}_{t}", bufs=2)
                    nc.vector.tensor_copy(thi, dacc[64:97, :])
                    ds = work.tile([33, 512], f32, tag="ds",
                                   name=f"ds{hp}_{t}", bufs=2)
                    nc.vector.tensor_add(ds, dacc[0:33, :], thi)
                    na, nb = make_norm(hp, t, ut, ds)
                    norm_a.append(na)
                    norm_b.append(nb)

            while wo_first:
                wo_first.popleft()()

            # ---- output projection, second q-half ----
            ops2 = wo_ops(range(4, NQ // P),
                          lambda t2, n: ((pmm, "mm") if (t2 * 2 + n) % 2 == 0
                                         else (psc, "sc")))
            # sc-tagged chains' first 3 MMs (c=0..2 don't read vals[3]) run
            # ahead of the final norm; their c=3 MM and the rest follow
            for j in (1, 3):
                for c in range(3):
                    ops2[j * MT + c]()
            while norm_a:
                norm_a.popleft()()
            while norm_b:
                norm_b.popleft()()
            for j in (1, 3):
                ops2[j * MT + 3]()
            for j in (0, 2):
                for c in range(MT):
                    ops2[j * MT + c]()
            for i in range(4 * MT, len(ops2)):
                ops2[i]()

    nc.compile()
    return nc


def _get_program():
    if "nc" not in _CACHE:
        _CACHE["nc"] = _build_program()
    return _CACHE["nc"]


def _swizzle_rows(a):
    """[D, C] -> [128, (D//128)*C]: chunk d's rows at free-offset d*C."""
    d128 = a.shape[0] // P
    return np.ascontiguousarray(
        a.reshape(d128, P, -1).transpose(1, 0, 2).reshape(P, -1)
    )


def _per_core_inputs(x, y, mask, W_kv, b_kv, W_q, b_q, W_o):
    """Build the 8 per-core input maps (host-preswizzled to SBUF layouts)."""
    in_maps = []
    mask_f = mask.astype(np.float32)
    for c in range(8):
        b, g = c // 2, c % 2
        gh = np.arange(g * 8, g * 8 + 8)
        k_cols = (gh[:, None] * 2 * HD + np.arange(HD)[None, :]).ravel()
        v_cols = k_cols + HD
        q_cols = slice(g * 512, (g + 1) * 512)

        xT = x[b].T                                  # [D, NKV]
        # xT dram [512, 4096]: row qb*128+p, col d*512+cc =
        #   xT[d*128+p, qb*512+cc]
        xr = xT.reshape(DC, P, 4, 512)               # [d, p, qb, cc]
        x_host = np.ascontiguousarray(
            xr.transpose(2, 1, 0, 3).reshape(512, 4096)).astype(BF)
        yT = y[b].T                                  # [D, NQ]
        y_host = _swizzle_rows(yT).astype(BF)        # [128, 8*1024]
        mT = mask_f[b].T                             # [NKV, NQ]
        m_host = _swizzle_rows(mT).astype(BF)        # [128, 16*1024]
        wk_host = _swizzle_rows(W_kv[:, k_cols]).astype(BF)
        wv_host = _swizzle_rows(W_kv[:, v_cols]).astype(BF)
        wq_host = _swizzle_rows(W_q[:, q_cols]).astype(BF)
        wo_host = _swizzle_rows(W_o[q_cols, :]).astype(BF)
        bk_host = np.ascontiguousarray(
            b_kv[k_cols].astype(np.float32).reshape(MT, P).T)
        bq_host = np.ascontiguousarray(
            b_q[np.arange(g * 512, (g + 1) * 512)]
            .astype(np.float32).reshape(MT, P).T)
        in_maps.append({
            "xT": x_host, "yT": y_host, "maskT": m_host,
            "wk": wk_host, "wv": wv_host, "wq": wq_host, "wo": wo_host,
            "bk": bk_host, "bq": bq_host,
        })
    return in_maps


def kernel(x, y, mask, W_kv, b_kv, W_q, b_q, W_o, b_o):
    from concourse import bass_utils

    x = np.asarray(x, np.float32)
    y = np.asarray(y, np.float32)
    mask = np.asarray(mask)
    W_kv = np.asarray(W_kv, np.float32)
    b_kv = np.asarray(b_kv, np.float32)
    W_q = np.asarray(W_q, np.float32)
    b_q = np.asarray(b_q, np.float32)
    W_o = np.asarray(W_o, np.float32)
    b_o = np.asarray(b_o, np.float32)

    nc = _get_program()
    in_maps = _per_core_inputs(x, y, mask, W_kv, b_kv, W_q, b_q, W_o)
    res = bass_utils.run_bass_kernel_spmd(nc, in_maps, core_ids=list(range(8)))

    # b_v folds into a constant row: attn rows sum to 1, so each head adds
    # b_v_h @ W_o_h to every output row; b_o adds on top.
    v_cols_all = (np.arange(H)[:, None] * 2 * HD + HD
                  + np.arange(HD)[None, :]).ravel()
    const_row = b_kv[v_cols_all].astype(np.float32) @ W_o + b_o

    out = np.empty((B, NQ, D), np.float32)
    for b in range(B):
        out[b] = (res.results[2 * b]["out"].astype(np.float32)
                  + res.results[2 * b + 1]["out"].astype(np.float32)
                  + const_row)
    return out


if __name__ == "__main__":
    import reference

    inputs = {k: np.asarray(v) for k, v in reference.setup_inputs().items()}
    got = kernel(**inputs)
    exp = np.asarray(reference.reference(**inputs))
    err = np.abs(got - exp)
    print("absmax rel err:", err.max() / np.abs(exp).max())


# revision 16
# speedup vs baseline: 1.0415x; 1.0415x over previous
"""Multi-head cross-attention Trainium2 Bass kernel, SPMD over 8 NeuronCores.

Sharding: core c handles batch b = c//2 and head group g = c%2 (8 of 16 heads).
Each core computes a partial output projection (its heads' W_o rows); the host
sums the two partials per batch element.

Device pipeline per core (all matmuls bf16 with fp32 PSUM accumulation):
  kT = (Wk^T x^T)          [512 hd, 2048 kseq]   (per-partition bias b_k)
  v  = (x Wv)              [2048 kseq, 8*64]
  qT = (Wq^T y^T)          [512 hd, 1024 q]      (per-partition bias b_q)
  per (head-pair, q-tile, k-chunk):
      S^T[k, q|q'] = kT_h^T-chunk @ qT_h for both heads of the pair
        (K=64 row-tiled at partitions 0/64 -> concurrent in the PE array)
      em = exp(0.125 * S^T)  (one ACT op per pair)
      em *= maskT            (one DVE mul per pair, head-broadcast)
      attnV: col-tiled pair into ONE psum bank: h0 -> vacc[0:64],
        h1 -> vacc[64:128]  (tile_position (0,0)/(0,64), concurrent)
      denominators: every 2nd kc, a 4-way col-tiled batch of ones-matmuls
        (strips at psum partitions 0/32/64/96) accumulates sum_k em for
        (even-kc h0, even h1, odd h0, odd h1) -> 2 kc amortized per 512 cyc
  normalize per block: dsum rows -> reciprocal_approx_fast -> bf16 ->
      one K=2 selector matmul (E2) broadcasts [2,512] over the 128
      partitions -> one [128,512] DVE mul into vals
  out_partial = vals^T-chunks @ Wo-rows  -> [1024 q, 1024 D]
Load phase: inputs are host-preswizzled to SBUF layout so each tensor is one
large contiguous DMA (the per-DMA issue cost on the Sync queue is ~0.6us;
many small DMAs made the old load phase issue-bound).  ~30 junk matmuls on a
zeroed tile warm the PE clock (HAM) during the DMA window.
b_v and b_o fold into a host-side constant row (attn rows sum to 1).
"""

import sys
from collections import deque

import numpy as np
import ml_dtypes

if "/opt/trn_rl_repo" not in sys.path:
    sys.path.insert(0, "/opt/trn_rl_repo")

BF = ml_dtypes.bfloat16

B, NKV, NQ, D, H = 4, 2048, 1024, 1024, 16
HD = D // H          # 64
NHL = 8              # heads per core (local)
P = 128
DC = D // P          # 8 contraction chunks over model dim
KC = NKV // P        # 16 key-seq chunks
QT = NQ // 512       # 2 q tiles of 512 for attention
MT = 4               # hd-dim chunks of kT/qT (512/128)

_CACHE = {}


def _build_program():
    import concourse.bass as bass
    import concourse.mybir as mybir
    import concourse.tile as tile
    from concourse import bacc

    f32 = mybir.dt.float32
    bf16 = mybir.dt.bfloat16

    nc = bacc.Bacc(
        "TRN2", target_bir_lowering=False, debug=False, num_devices=8
    )

    # host-preswizzled inputs: each is one contiguous DMA into its SBUF tile
    xT_d = nc.dram_tensor("xT", [512, 4096], bf16, kind="ExternalInput").ap()
    yT_d = nc.dram_tensor("yT", [P, DC * NQ], bf16, kind="ExternalInput").ap()
    maskT_d = nc.dram_tensor(
        "maskT", [P, KC * NQ], bf16, kind="ExternalInput"
    ).ap()
    wk_d = nc.dram_tensor("wk", [P, DC * 512], bf16, kind="ExternalInput").ap()
    wv_d = nc.dram_tensor("wv", [P, DC * 512], bf16, kind="ExternalInput").ap()
    wq_d = nc.dram_tensor("wq", [P, DC * 512], bf16, kind="ExternalInput").ap()
    wo_d = nc.dram_tensor("wo", [P, MT * D], bf16, kind="ExternalInput").ap()
    bk_d = nc.dram_tensor("bk", [P, MT], f32, kind="ExternalInput").ap()
    bq_d = nc.dram_tensor("bq", [P, MT], f32, kind="ExternalInput").ap()
    out_d = nc.dram_tensor("out", [NQ, D], bf16, kind="ExternalOutput").ap()

    Exp = mybir.ActivationFunctionType.Exp

    with tile.TileContext(nc) as tc:
        with (
            tc.tile_pool(name="persist", bufs=1) as persist,
            tc.tile_pool(name="work", bufs=3) as work,
            tc.tile_pool(name="empool", bufs=6) as empool,
            tc.tile_pool(name="pmm", bufs=1, space="PSUM") as pmm,
            tc.tile_pool(name="pvac", bufs=2, space="PSUM") as pvac,
            tc.tile_pool(name="pdac", bufs=1, space="PSUM") as pdac,
            tc.tile_pool(name="psc", bufs=2, space="PSUM") as psc,
        ):
            # ---- persistent SBUF tiles ----
            wk_sb = persist.tile([P, DC * 512], bf16, tag="wk", name="wk")
            wv_sb = persist.tile([P, DC * 512], bf16, tag="wv", name="wv")
            wq_sb = persist.tile([P, DC * 512], bf16, tag="wq", name="wq")
            wo_sb = persist.tile([P, MT * D], bf16, tag="wo", name="wo")
            xT_sb = persist.tile([P, DC * NKV], bf16, tag="xT", name="xT")
            yT_sb = persist.tile([P, DC * NQ], bf16, tag="yT", name="yT")
            maskT_sb = persist.tile(
                [P, KC * NQ], bf16, tag="mT", name="mT"
            )
            bk_sb = persist.tile([P, MT], f32, tag="bk", name="bk")
            bq_sb = persist.tile([P, MT], f32, tag="bq", name="bq")

            def xs(d, lo, hi):     # xT chunk d, key-cols lo:hi
                return xT_sb[:, d * NKV + lo:d * NKV + hi]

            def ys(d, lo, hi):
                return yT_sb[:, d * NQ + lo:d * NQ + hi]

            # warm tile + constants
            warm = persist.tile([P, 256], bf16, tag="warm", name="warm")
            nc.gpsimd.memset(warm, 0.0)
            ones_col = persist.tile([P, 1], bf16, tag="onesc", name="onesc")
            nc.gpsimd.memset(ones_col, 1.0)
            # selector for the denominator broadcast: row 0 -> parts 0-63,
            # row 32 -> parts 64-127 (K=33 matmul; rows 1-31 are zero)
            e33_sb = persist.tile([33, P], bf16, tag="e33", name="e33")
            nc.gpsimd.memset(e33_sb, 0.0)
            nc.gpsimd.memset(e33_sb[0:1, 0:HD], 1.0)
            nc.gpsimd.memset(e33_sb[32:33, HD:2 * HD], 1.0)

            # ---- HAM warmup: junk matmuls on the zeroed tile ----
            wps = psc.tile([P, 1024], f32, tag="sc", name="warmps")
            for i in range(48):
                nc.tensor.matmul(
                    wps[:, 0:256], lhsT=warm[:, 0:P], rhs=warm,
                    start=True, stop=True,
                )

            # ---- input DMAs: one per tensor (xT by column-quarters) ----
            nc.sync.dma_start(wk_sb, wk_d)
            nc.sync.dma_start(bk_sb, bk_d)
            nc.sync.dma_start(bq_sb, bq_d)
            x3 = xT_sb.rearrange("p (d c) -> p d c", d=DC)

            def load_xq(qb):
                nc.sync.dma_start(
                    x3[:, :, qb * 512:(qb + 1) * 512],
                    xT_d[qb * P:(qb + 1) * P, :].rearrange(
                        "p (d c) -> p d c", d=DC),
                )
            load_xq(0)
            nc.sync.dma_start(wv_sb, wv_d)
            for qb in range(1, 4):
                load_xq(qb)
            nc.sync.dma_start(wq_sb, wq_d)
            nc.sync.dma_start(yT_sb, yT_d)
            for mq in range(4):
                nc.sync.dma_start(
                    maskT_sb[:, mq * 4096:(mq + 1) * 4096],
                    maskT_d[:, mq * 4096:(mq + 1) * 4096],
                )
            nc.sync.dma_start(wo_sb, wo_d)

            kT_sb = [
                persist.tile([P, NKV], bf16, tag=f"kT{m}", name=f"kT{m}")
                for m in range(MT)
            ]
            qT_sb = [
                persist.tile([P, NQ], bf16, tag=f"qT{m}", name=f"qT{m}")
                for m in range(MT)
            ]
            v_sb = [
                persist.tile([P, NHL * HD], bf16, tag=f"v{i}", name=f"v{i}")
                for i in range(KC)
            ]
            vals_sb = [
                persist.tile([P, NQ], bf16, tag=f"vals{c}", name=f"vals{c}")
                for c in range(MT)
            ]

            # ---- projection op queues (closures; chain tail adds bias) ----
            def proj_ops(m, which, ns=None, pool=None):
                w_sb, dst, bias, ncols = (
                    (wk_sb, kT_sb, bk_sb, NKV) if which == "k"
                    else (wq_sb, qT_sb, bq_sb, NQ)
                )
                src = xs if which == "k" else ys
                pool_ = pool if pool is not None else pmm
                tag = "sc" if pool_ is psc else "mm"
                ops = []
                hold = {}
                nlist = list(range(ncols // 512) if ns is None else ns)
                for n in nlist:
                    for d in range(DC):
                        def op(m=m, n=n, d=d, w_sb=w_sb, dst=dst, bias=bias,
                               which=which, src=src, pool_=pool_, tag=tag):
                            if d == 0:
                                hold[n] = pool_.tile(
                                    [P, 512] if tag == "mm" else [P, 1024],
                                    f32, tag=tag, name=f"pj{which}{m}_{n}"
                                )
                            ps = hold[n][:, 0:512]
                            nc.tensor.matmul(
                                ps,
                                lhsT=w_sb[:, d * 512 + m * P:
                                          d * 512 + (m + 1) * P],
                                rhs=src(d, n * 512, (n + 1) * 512),
                                start=(d == 0),
                                stop=(d == DC - 1),
                            )
                            if d == DC - 1:
                                nc.vector.tensor_scalar_add(
                                    dst[m][:, n * 512:(n + 1) * 512],
                                    ps, bias[:, m:m + 1]
                                )
                        ops.append(op)
                return ops

            def v_ops(i, pool_):
                tag = "sc" if pool_ is psc else "mm"
                ops = []
                hold = {}

                def mk(d, i=i, pool_=pool_, tag=tag):
                    def op(d=d):
                        if d == 0:
                            hold[0] = pool_.tile(
                                [P, 512] if tag == "mm" else [P, 1024],
                                f32, tag=tag, name=f"ps_v{i}"
                            )
                        nc.tensor.matmul(
                            hold[0][:, 0:512],
                            lhsT=xs(d, i * P, (i + 1) * P),
                            rhs=wv_sb[:, d * 512:(d + 1) * 512],
                            start=(d == 0),
                            stop=(d == DC - 1),
                        )
                        if d == DC - 1:
                            nc.vector.tensor_copy(v_sb[i], hold[0][:, 0:512])
                    return op
                for d in range(DC):
                    ops.append(mk(d))
                return ops

            # upfront: kT chunk-0 chains as their xT quarters land, early v
            # chains, then qT chunk-0 — attention starts ASAP so ACT/DVE ramp
            for op in proj_ops(0, "k", ns=[0], pool=psc):
                op()
            for i in (0, 1, 2, 3):
                for op in v_ops(i, psc):
                    op()
            for op in proj_ops(0, "k", ns=[1], pool=psc):
                op()
            for i in (4, 5, 6, 7):
                for op in v_ops(i, psc):
                    op()
            for op in proj_ops(0, "k", ns=[2], pool=psc):
                op()
            for i in (8, 9):
                for op in v_ops(i, psc):
                    op()
            for op in proj_ops(0, "k", ns=[3], pool=psc):
                op()
            for i in (10, 11):
                for op in v_ops(i, psc):
                    op()
            for op in proj_ops(0, "q", pool=psc):
                op()

            # deferred work, drained inside the attention loops:
            # hp0 gets the v tail + chunk-1 projections, etc.
            pending = {
                0: deque(
                    v_ops(12, pmm) + v_ops(13, pmm) + v_ops(14, pmm)
                    + v_ops(15, pmm)
                    + proj_ops(1, "k") + proj_ops(1, "q")
                ),
                1: deque(proj_ops(2, "k") + proj_ops(2, "q")),
                2: deque(proj_ops(3, "k") + proj_ops(3, "q")),
                3: deque(),
            }

            # ---- output-projection chains (closures) ----
            def wo_ops(t2s, pool_pick, copy_eng="scalar"):
                ops = []
                hold = {}
                for t2 in t2s:
                    for n in range(D // 512):
                        for c in range(MT):
                            def op(t2=t2, n=n, c=c):
                                if c == 0:
                                    pool, tag = pool_pick(t2, n)
                                    hold[(t2, n)] = pool.tile(
                                        [P, 512] if tag == "mm"
                                        else [P, 1024],
                                        f32, tag=tag, name=f"ps_o{t2}_{n}"
                                    )
                                ps_o = hold[(t2, n)][:, 0:512]
                                nc.tensor.matmul(
                                    ps_o,
                                    lhsT=vals_sb[c][:, t2 * P:(t2 + 1) * P],
                                    rhs=wo_sb[:, c * D + n * 512:
                                              c * D + (n + 1) * 512],
                                    start=(c == 0),
                                    stop=(c == MT - 1),
                                )
                                if c == MT - 1:
                                    ot = work.tile(
                                        [P, 512], bf16, tag="ot",
                                        name=f"ot{t2}_{n}", bufs=3
                                    )
                                    if copy_eng == "scalar":
                                        nc.scalar.copy(ot, ps_o)
                                    else:
                                        nc.vector.tensor_copy(ot, ps_o)
                                    nc.sync.dma_start(
                                        out_d[t2 * P:(t2 + 1) * P,
                                              n * 512:(n + 1) * 512], ot
                                    )
                            ops.append(op)
                return ops

            wo_first = deque(wo_ops(range(0, 4), lambda t2, n: (pmm, "mm")))

            # ---- attention ----
            norm_a = deque()
            norm_b = deque()

            def make_norm(hp, t, ut, ds):
                qs = slice(t * 512, (t + 1) * 512)

                def na():
                    rec = work.tile([33, 512], f32, tag="rec",
                                    name=f"rec{hp}_{t}", bufs=2)
                    nc.vector.reciprocal_approx_fast(rec, ds)
                    rb = work.tile([33, 512], bf16, tag="rb",
                                   name=f"rb{hp}_{t}", bufs=2)
                    nc.vector.tensor_copy(rb, rec)
                    na.rb = rb

                def nb():
                    bps = pmm.tile([P, 512], f32, tag="mm",
                                   name=f"bps{hp}_{t}")
                    nc.tensor.matmul(
                        bps, lhsT=e33_sb, rhs=na.rb, start=True, stop=True
                    )
                    nc.vector.tensor_mul(vals_sb[hp][:, qs], ut, bps)
                return na, nb

            for hp in range(MT):
                h0, h1 = 2 * hp, 2 * hp + 1
                q = pending[hp]
                for t in range(QT):
                    qs = slice(t * 512, (t + 1) * 512)
                    slots_left = (QT - t) * KC
                    vacc = pvac.tile([P, 512], f32, tag="vacc",
                                     name=f"vacc{hp}_{t}")
                    dacc = pdac.tile([P, 512], f32, tag="dacc",
                                     name=f"dacc{hp}_{t}")
                    ems = {}
                    # software-pipelined: attnV/denom for kc-1 are emitted
                    # one iteration behind scores/exp/mask(kc), so the PE
                    # never sits on the exp->mask latency; pending drains
                    # go AFTER the attention ops so mask keeps DVE priority
                    for it in range(KC + 1):
                        if it < KC:
                            kc = it
                            sp2 = psc.tile(
                                [P, 1024], f32, tag="sc",
                                name=f"sp{hp}_{t}_{kc}"
                            )
                            for a in range(2):
                                po = a * HD
                                nc.tensor.matmul(
                                    sp2[:, a * 512:(a + 1) * 512],
                                    lhsT=kT_sb[hp][po:po + HD,
                                                   kc * P:(kc + 1) * P],
                                    rhs=qT_sb[hp][po:po + HD, qs],
                                    start=True,
                                    stop=True,
                                )
                            em2 = empool.tile(
                                [P, 1024], bf16, tag="em",
                                name=f"em{hp}_{t}_{kc}"
                            )
                            nc.scalar.activation(em2, sp2, Exp, scale=0.125)
                            mb = (maskT_sb[:, kc * NQ + t * 512:
                                           kc * NQ + (t + 1) * 512]
                                  .rearrange("p (o q) -> p o q", o=1)
                                  .broadcast_to([P, 2, 512]))
                            em3 = em2.rearrange("p (o q) -> p o q", o=2)
                            nc.vector.tensor_mul(em3, em3, mb)
                            ems[kc] = em2
                        if it >= 1:
                            kd = it - 1
                            emd = ems[kd]
                            # attnV: col-tiled pair into one bank (disjoint
                            # partition ranges -> independent accum groups)
                            for a, h in enumerate((h0, h1)):
                                nc.tensor.matmul(
                                    vacc[a * HD:(a + 1) * HD, :],
                                    lhsT=v_sb[kd][:, h * HD:(h + 1) * HD],
                                    rhs=emd[:, a * 512:(a + 1) * 512],
                                    start=(kd == 0),
                                    stop=(kd == KC - 1),
                                )
                            # denominators: 4-way col-tiled ones-matmuls,
                            # strips (0,32,64,96) <- (even h0, even h1,
                            # odd h0, odd h1); accumulated over 8 batches
                            if kd % 2 == 1:
                                for j, (emt, a) in enumerate(
                                    ((ems[kd - 1], 0), (ems[kd - 1], 1),
                                     (emd, 0), (emd, 1))
                                ):
                                    nc.tensor.matmul(
                                        dacc[32 * j:32 * j + 1, :],
                                        lhsT=ones_col,
                                        rhs=emt[:, a * 512:(a + 1) * 512],
                                        start=(kd == 1),
                                        stop=(kd == KC - 1),
                                        tile_position=(0, 32 * j),
                                    )
                        if it < KC:
                            n_emit = -(-len(q) // slots_left)  # ceil
                            for _ in range(min(n_emit, len(q))):
                                q.popleft()()
                            slots_left -= 1
                            if it == 2 and norm_a:
                                norm_a.popleft()()
                            if it == 6 and norm_b:
                                norm_b.popleft()()
                            if hp == 3 and t == 1 and it >= 8:
                                for _ in range(4):
                                    if wo_first:
                                        wo_first.popleft()()
                    # block epilogue: free vacc/dacc fast, defer the rest.
                    # ds rows 0/32 = h0/h1 denominators (odd-kc strips at
                    # parts 64/96 copied down, added to even strips 0/32;
                    # rows 1-31 are junk and never read downstream)
                    ut = work.tile([P, 512], f32, tag="ut",
                                   name=f"ut{hp}_{t}", bufs=2)
                    nc.vector.tensor_copy(ut, vacc)
                    thi = work.tile([33, 512], f32, tag="thi",
                                    name=f"thi{hp}_{t}", bufs=2)
                    nc.vector.tensor_copy(thi, dacc[64:97, :])
                    ds = work.tile([33, 512], f32, tag="ds",
                                   name=f"ds{hp}_{t}", bufs=2)
                    nc.vector.tensor_add(ds, dacc[0:33, :], thi)
                    na, nb = make_norm(hp, t, ut, ds)
                    norm_a.append(na)
                    norm_b.append(nb)

            while wo_first:
                wo_first.popleft()()

            # ---- output projection, second q-half ----
            ops2 = wo_ops(range(4, NQ // P),
                          lambda t2, n: ((pmm, "mm") if (t2 * 2 + n) % 4 == 0
                                         else (psc, "sc")),
                          copy_eng="vector")
            # sc-tagged chains' first 3 MMs (c=0..2 don't read vals[3]) run
            # ahead of the final norm; their c=3 MM and the rest follow
            for j in (1, 3):
                for c in range(3):
                    ops2[j * MT + c]()
            while norm_a:
                norm_a.popleft()()
            while norm_b:
                norm_b.popleft()()
            for j in (1, 3):
                ops2[j * MT + 3]()
            for j in (0, 2):
                for c in range(MT):
                    ops2[j * MT + c]()
            for i in range(4 * MT, len(ops2)):
                ops2[i]()

    nc.compile()
    return nc


def _get_program():
    if "nc" not in _CACHE:
        _CACHE["nc"] = _build_program()
    return _CACHE["nc"]


def _swizzle_rows(a):
    """[D, C] -> [128, (D//128)*C]: chunk d's rows at free-offset d*C."""
    d128 = a.shape[0] // P
    return np.ascontiguousarray(
        a.reshape(d128, P, -1).transpose(1, 0, 2).reshape(P, -1)
    )


def _per_core_inputs(x, y, mask, W_kv, b_kv, W_q, b_q, W_o):
    """Build the 8 per-core input maps (host-preswizzled to SBUF layouts)."""
    in_maps = []
    mask_f = mask.astype(np.float32)
    for c in range(8):
        b, g = c // 2, c % 2
        gh = np.arange(g * 8, g * 8 + 8)
        k_cols = (gh[:, None] * 2 * HD + np.arange(HD)[None, :]).ravel()
        v_cols = k_cols + HD
        q_cols = slice(g * 512, (g + 1) * 512)

        xT = x[b].T                                  # [D, NKV]
        # xT dram [512, 4096]: row qb*128+p, col d*512+cc =
        #   xT[d*128+p, qb*512+cc]
        xr = xT.reshape(DC, P, 4, 512)               # [d, p, qb, cc]
        x_host = np.ascontiguousarray(
            xr.transpose(2, 1, 0, 3).reshape(512, 4096)).astype(BF)
        yT = y[b].T                                  # [D, NQ]
        y_host = _swizzle_rows(yT).astype(BF)        # [128, 8*1024]
        mT = mask_f[b].T                             # [NKV, NQ]
        m_host = _swizzle_rows(mT).astype(BF)        # [128, 16*1024]
        wk_host = _swizzle_rows(W_kv[:, k_cols]).astype(BF)
        wv_host = _swizzle_rows(W_kv[:, v_cols]).astype(BF)
        wq_host = _swizzle_rows(W_q[:, q_cols]).astype(BF)
        wo_host = _swizzle_rows(W_o[q_cols, :]).astype(BF)
        bk_host = np.ascontiguousarray(
            b_kv[k_cols].astype(np.float32).reshape(MT, P).T)
        bq_host = np.ascontiguousarray(
            b_q[np.arange(g * 512, (g + 1) * 512)]
            .astype(np.float32).reshape(MT, P).T)
        in_maps.append({
            "xT": x_host, "yT": y_host, "maskT": m_host,
            "wk": wk_host, "wv": wv_host, "wq": wq_host, "wo": wo_host,
            "bk": bk_host, "bq": bq_host,
        })
    return in_maps


def kernel(x, y, mask, W_kv, b_kv, W_q, b_q, W_o, b_o):
    from concourse import bass_utils

    x = np.asarray(x, np.float32)
    y = np.asarray(y, np.float32)
    mask = np.asarray(mask)
    W_kv = np.asarray(W_kv, np.float32)
    b_kv = np.asarray(b_kv, np.float32)
    W_q = np.asarray(W_q, np.float32)
    b_q = np.asarray(b_q, np.float32)
    W_o = np.asarray(W_o, np.float32)
    b_o = np.asarray(b_o, np.float32)

    nc = _get_program()
    in_maps = _per_core_inputs(x, y, mask, W_kv, b_kv, W_q, b_q, W_o)
    res = bass_utils.run_bass_kernel_spmd(nc, in_maps, core_ids=list(range(8)))

    # b_v folds into a constant row: attn rows sum to 1, so each head adds
    # b_v_h @ W_o_h to every output row; b_o adds on top.
    v_cols_all = (np.arange(H)[:, None] * 2 * HD + HD
                  + np.arange(HD)[None, :]).ravel()
    const_row = b_kv[v_cols_all].astype(np.float32) @ W_o + b_o

    out = np.empty((B, NQ, D), np.float32)
    for b in range(B):
        out[b] = (res.results[2 * b]["out"].astype(np.float32)
                  + res.results[2 * b + 1]["out"].astype(np.float32)
                  + const_row)
    return out


if __name__ == "__main__":
    import reference

    inputs = {k: np.asarray(v) for k, v in reference.setup_inputs().items()}
    got = kernel(**inputs)
    exp = np.asarray(reference.reference(**inputs))
    err = np.abs(got - exp)
    print("absmax rel err:", err.max() / np.abs(exp).max())


# revision 27
# speedup vs baseline: 1.1387x; 1.0933x over previous
"""Multi-head cross-attention Trainium2 Bass kernel, SPMD over 8 NeuronCores.

Sharding: core c handles batch b = c//2 and head group g = c%2 (8 of 16 heads).
Each core computes a partial output projection (its heads' W_o rows); the host
sums the two partials per batch element.

Device pipeline per core (all matmuls bf16 with fp32 PSUM accumulation):
  kT = (Wk^T x^T)          [512 hd, 2048 kseq]   (per-partition bias b_k)
  v  = (x Wv)              [2048 kseq, 8*64]
  qT = (Wq^T y^T)          [512 hd, 1024 q]      (per-partition bias b_q)
  per (head-pair, q-tile, k-chunk):
      S^T[k, q|q'] = kT_h^T-chunk @ qT_h for both heads of the pair
        (K=64 row-tiled at partitions 0/64 -> concurrent in the PE array)
      em = exp(0.125 * S^T)  (one ACT op per pair)
      em *= maskT            (one DVE mul per pair, head-broadcast)
      attnV: col-tiled pair into ONE psum bank: h0 -> vacc[0:64],
        h1 -> vacc[64:128]  (tile_position (0,0)/(0,64), concurrent)
      denominators: every 2nd kc, a 4-way col-tiled batch of ones-matmuls
        (strips at psum partitions 0/32/64/96) accumulates sum_k em for
        (even-kc h0, even h1, odd h0, odd h1) -> 2 kc amortized per 512 cyc
  normalize per block: dsum rows -> reciprocal_approx_fast -> bf16 ->
      one K=2 selector matmul (E2) broadcasts [2,512] over the 128
      partitions -> one [128,512] DVE mul into vals
  out_partial = vals^T-chunks @ Wo-rows  -> [1024 q, 1024 D]
Load phase: inputs are host-preswizzled to SBUF layout so each tensor is one
large contiguous DMA (the per-DMA issue cost on the Sync queue is ~0.6us;
many small DMAs made the old load phase issue-bound).  ~30 junk matmuls on a
zeroed tile warm the PE clock (HAM) during the DMA window.
b_v and b_o fold into a host-side constant row (attn rows sum to 1).
"""

import sys
from collections import deque

import numpy as np
import ml_dtypes

if "/opt/trn_rl_repo" not in sys.path:
    sys.path.insert(0, "/opt/trn_rl_repo")

BF = ml_dtypes.bfloat16

B, NKV, NQ, D, H = 4, 2048, 1024, 1024, 16
HD = D // H          # 64
NHL = 8              # heads per core (local)
P = 128
DC = D // P          # 8 contraction chunks over model dim
KC = NKV // P        # 16 key-seq chunks
QT = NQ // 512       # 2 q tiles of 512 for attention
MT = 4               # hd-dim chunks of kT/qT (512/128)

_CACHE = {}


def _build_program():
    import concourse.bass as bass
    import concourse.mybir as mybir
    import concourse.tile as tile
    from concourse import bacc

    f32 = mybir.dt.float32
    bf16 = mybir.dt.bfloat16

    nc = bacc.Bacc(
        "TRN2", target_bir_lowering=False, debug=False, num_devices=8
    )

    # host-preswizzled inputs: each is one contiguous DMA into its SBUF tile
    xT_d = nc.dram_tensor("xT", [512, 4096], bf16, kind="ExternalInput").ap()
    yT_d = nc.dram_tensor("yT", [P, DC * NQ], bf16, kind="ExternalInput").ap()
    maskT_d = nc.dram_tensor(
        "maskT", [P, KC * NQ], bf16, kind="ExternalInput"
    ).ap()
    wk_d = nc.dram_tensor("wk", [P, DC * 512], bf16, kind="ExternalInput").ap()
    wv_d = nc.dram_tensor("wv", [P, DC * 512], bf16, kind="ExternalInput").ap()
    wq_d = nc.dram_tensor("wq", [P, DC * 512], bf16, kind="ExternalInput").ap()
    wo_d = nc.dram_tensor("wo", [P, MT * D], bf16, kind="ExternalInput").ap()
    bk_d = nc.dram_tensor("bk", [P, MT], f32, kind="ExternalInput").ap()
    bq_d = nc.dram_tensor("bq", [P, MT], f32, kind="ExternalInput").ap()
    out_d = nc.dram_tensor("out", [NQ, D], bf16, kind="ExternalOutput").ap()

    Exp = mybir.ActivationFunctionType.Exp

    with tile.TileContext(nc) as tc:
        with (
            tc.tile_pool(name="persist", bufs=1) as persist,
            tc.tile_pool(name="work", bufs=3) as work,
            tc.tile_pool(name="empool", bufs=6) as empool,
            tc.tile_pool(name="pmm", bufs=2, space="PSUM") as pmm,
            tc.tile_pool(name="pacc", bufs=2, space="PSUM") as pacc,
            tc.tile_pool(name="psc", bufs=2, space="PSUM") as psc,
        ):
            # ---- persistent SBUF tiles ----
            wk_sb = persist.tile([P, DC * 512], bf16, tag="wk", name="wk")
            wv_sb = persist.tile([P, DC * 512], bf16, tag="wv", name="wv")
            wq_sb = persist.tile([P, DC * 512], bf16, tag="wq", name="wq")
            wo_sb = persist.tile([P, MT * D], bf16, tag="wo", name="wo")
            xT_sb = persist.tile([P, DC * NKV], bf16, tag="xT", name="xT")
            yT_sb = persist.tile([P, DC * NQ], bf16, tag="yT", name="yT")
            maskT_sb = persist.tile(
                [P, KC * NQ], bf16, tag="mT", name="mT"
            )
            bk_sb = persist.tile([P, MT], f32, tag="bk", name="bk")
            bq_sb = persist.tile([P, MT], f32, tag="bq", name="bq")

            def xs(d, lo, hi):     # xT chunk d, key-cols lo:hi
                return xT_sb[:, d * NKV + lo:d * NKV + hi]

            def ys(d, lo, hi):
                return yT_sb[:, d * NQ + lo:d * NQ + hi]

            # warm tile + constants
            warm = persist.tile([P, 256], bf16, tag="warm", name="warm")
            nc.gpsimd.memset(warm, 0.0)

            ones_row = persist.tile([1, HD], bf16, tag="onesr", name="onesr")
            nc.gpsimd.memset(ones_row, 1.0)

            # ---- HAM warmup: junk matmuls on the zeroed tile ----
            wps = psc.tile([P, 1024], f32, tag="sc", name="warmps")
            for i in range(48):
                nc.tensor.matmul(
                    wps[:, 0:256], lhsT=warm[:, 0:P], rhs=warm,
                    start=True, stop=True,
                )

            # ---- input DMAs: one per tensor (xT by column-quarters) ----
            nc.sync.dma_start(wk_sb, wk_d)
            nc.sync.dma_start(bk_sb, bk_d)
            nc.sync.dma_start(bq_sb, bq_d)
            x3 = xT_sb.rearrange("p (d c) -> p d c", d=DC)

            def load_xq(qb):
                nc.sync.dma_start(
                    x3[:, :, qb * 512:(qb + 1) * 512],
                    xT_d[qb * P:(qb + 1) * P, :].rearrange(
                        "p (d c) -> p d c", d=DC),
                )
            load_xq(0)
            nc.sync.dma_start(wv_sb, wv_d)
            for qb in range(1, 4):
                load_xq(qb)
            nc.sync.dma_start(wq_sb, wq_d)
            nc.sync.dma_start(yT_sb, yT_d)
            for mq in range(4):
                nc.sync.dma_start(
                    maskT_sb[:, mq * 4096:(mq + 1) * 4096],
                    maskT_d[:, mq * 4096:(mq + 1) * 4096],
                )
            nc.sync.dma_start(wo_sb, wo_d)

            kT_sb = [
                persist.tile([P, NKV], bf16, tag=f"kT{m}", name=f"kT{m}")
                for m in range(MT)
            ]
            qT_sb = [
                persist.tile([P, NQ], bf16, tag=f"qT{m}", name=f"qT{m}")
                for m in range(MT)
            ]
            v_sb = [
                persist.tile([P, NHL * 65], bf16, tag=f"v{i}", name=f"v{i}")
                for i in range(KC)
            ]
            for i in range(KC):
                nc.gpsimd.memset(
                    v_sb[i].rearrange("p (h c) -> p h c", c=65)[:, :, 64:65],
                    1.0,
                )
            vals_sb = [
                persist.tile([P, NQ], bf16, tag=f"vals{c}", name=f"vals{c}")
                for c in range(MT)
            ]

            # ---- projection op queues (closures; chain tail adds bias) ----
            def proj_ops(m, which, ns=None, pool=None):
                w_sb, dst, bias, ncols = (
                    (wk_sb, kT_sb, bk_sb, NKV) if which == "k"
                    else (wq_sb, qT_sb, bq_sb, NQ)
                )
                src = xs if which == "k" else ys
                pool_ = pool if pool is not None else pmm
                tag = "sc" if pool_ is psc else "mm"
                ops = []
                hold = {}
                nlist = list(range(ncols // 512) if ns is None else ns)
                for n in nlist:
                    for d in range(DC):
                        def op(m=m, n=n, d=d, w_sb=w_sb, dst=dst, bias=bias,
                               which=which, src=src, pool_=pool_, tag=tag):
                            if d == 0:
                                hold[n] = pool_.tile(
                                    [P, 512] if tag == "mm" else [P, 1024],
                                    f32, tag=tag, name=f"pj{which}{m}_{n}"
                                )
                            ps = hold[n][:, 0:512]
                            nc.tensor.matmul(
                                ps,
                                lhsT=w_sb[:, d * 512 + m * P:
                                          d * 512 + (m + 1) * P],
                                rhs=src(d, n * 512, (n + 1) * 512),
                                start=(d == 0),
                                stop=(d == DC - 1),
                            )
                            if d == DC - 1:
                                nc.vector.tensor_scalar_add(
                                    dst[m][:, n * 512:(n + 1) * 512],
                                    ps, bias[:, m:m + 1]
                                )
                        ops.append(op)
                return ops

            def v_ops(i, pool_):
                tag = "sc" if pool_ is psc else "mm"
                ops = []
                hold = {}

                def mk(d, i=i, pool_=pool_, tag=tag):
                    def op(d=d):
                        if d == 0:
                            hold[0] = pool_.tile(
                                [P, 512] if tag == "mm" else [P, 1024],
                                f32, tag=tag, name=f"ps_v{i}"
                            )
                        nc.tensor.matmul(
                            hold[0][:, 0:512],
                            lhsT=xs(d, i * P, (i + 1) * P),
                            rhs=wv_sb[:, d * 512:(d + 1) * 512],
                            start=(d == 0),
                            stop=(d == DC - 1),
                        )
                        if d == DC - 1:
                            v3 = v_sb[i].rearrange("p (h c) -> p h c", c=65)
                            nc.vector.tensor_copy(
                                v3[:, :, 0:64],
                                hold[0][:, 0:512].rearrange(
                                    "p (h c) -> p h c", c=64),
                            )
                    return op
                for d in range(DC):
                    ops.append(mk(d))
                return ops

            # upfront: kT chunk-0 chains as their xT quarters land, early v
            # chains, then qT chunk-0 — attention starts ASAP so ACT/DVE ramp
            for op in proj_ops(0, "k", ns=[0], pool=psc):
                op()
            for i in (0, 1, 2, 3):
                for op in v_ops(i, psc):
                    op()
            for op in proj_ops(0, "k", ns=[1], pool=psc):
                op()
            for i in (4, 5, 6, 7):
                for op in v_ops(i, psc):
                    op()
            for op in proj_ops(0, "k", ns=[2], pool=psc):
                op()
            for i in (8, 9):
                for op in v_ops(i, psc):
                    op()
            for op in proj_ops(0, "k", ns=[3], pool=psc):
                op()
            for i in (10, 11):
                for op in v_ops(i, psc):
                    op()
            for op in proj_ops(0, "q", pool=psc):
                op()

            # deferred work, drained inside the attention loops:
            # hp0 gets the v tail + chunk-1 projections, etc.
            pending = {
                0: deque(
                    v_ops(12, pmm) + v_ops(13, pmm) + v_ops(14, pmm)
                    + v_ops(15, pmm)
                    + proj_ops(1, "k") + proj_ops(1, "q")
                ),
                1: deque(proj_ops(2, "k") + proj_ops(2, "q")),
                2: deque(proj_ops(3, "k") + proj_ops(3, "q")),
                3: deque(),
            }

            # ---- output-projection chains (closures) ----
            def wo_ops(t2s, pool_pick, copy_eng="scalar"):
                ops = []
                hold = {}
                for t2 in t2s:
                    for n in range(D // 512):
                        for c in range(MT):
                            def op(t2=t2, n=n, c=c):
                                if c == 0:
                                    pool, tag = pool_pick(t2, n)
                                    hold[(t2, n)] = pool.tile(
                                        [P, 512] if tag == "mm"
                                        else [P, 1024],
                                        f32, tag=tag, name=f"ps_o{t2}_{n}"
                                    )
                                ps_o = hold[(t2, n)][:, 0:512]
                                nc.tensor.matmul(
                                    ps_o,
                                    lhsT=vals_sb[c][:, t2 * P:(t2 + 1) * P],
                                    rhs=wo_sb[:, c * D + n * 512:
                                              c * D + (n + 1) * 512],
                                    start=(c == 0),
                                    stop=(c == MT - 1),
                                )
                                if c == MT - 1:
                                    ot = work.tile(
                                        [P, 512], bf16, tag="ot",
                                        name=f"ot{t2}_{n}", bufs=3
                                    )
                                    if copy_eng == "scalar":
                                        nc.scalar.copy(ot, ps_o)
                                    else:
                                        nc.vector.tensor_copy(ot, ps_o)
                                    nc.sync.dma_start(
                                        out_d[t2 * P:(t2 + 1) * P,
                                              n * 512:(n + 1) * 512], ot
                                    )
                            ops.append(op)
                return ops

            wo_first = deque(wo_ops(range(0, 4), lambda t2, n: (pmm, "mm")))

            # ---- attention ----
            norm_a = deque()
            norm_b = deque()

            def make_norm(hp, t, ut, r0, r1):
                qs = slice(t * 512, (t + 1) * 512)

                def na():
                    rbs = []
                    for h, r in ((0, r0), (1, r1)):
                        rb = work.tile([1, 512], bf16, tag=f"rb{h}",
                                       name=f"rb{h}_{hp}_{t}", bufs=2)
                        nc.vector.tensor_copy(rb, r)
                        rbs.append(rb)
                    na.rbs = rbs

                def nb():
                    bps = pmm.tile([P, 512], f32, tag="mm",
                                   name=f"bps{hp}_{t}")
                    for a in range(2):
                        nc.tensor.matmul(
                            bps[a * HD:(a + 1) * HD, :],
                            lhsT=ones_row, rhs=na.rbs[a],
                            start=True, stop=True,
                        )
                    nc.vector.tensor_mul(vals_sb[hp][:, qs], ut, bps)
                return na, nb

            for hp in range(MT):
                h0, h1 = 2 * hp, 2 * hp + 1
                q = pending[hp]
                for t in range(QT):
                    qs = slice(t * 512, (t + 1) * 512)
                    slots_left = (QT - t) * KC
                    accs = [
                        pacc.tile([65, 512], f32, tag="acc",
                                  name=f"acc{h}_{t}")
                        for h in (h0, h1)
                    ]
                    ems = {}
                    # software-pipelined: attnV/denom for kc-1 are emitted
                    # one iteration behind scores/exp/mask(kc), so the PE
                    # never sits on the exp->mask latency; pending drains
                    # go AFTER the attention ops so mask keeps DVE priority
                    for it in range(KC + 1):
                        if it < KC:
                            kc = it
                            sp2 = psc.tile(
                                [P, 1024], f32, tag="sc",
                                name=f"sp{hp}_{t}_{kc}"
                            )
                            for a in range(2):
                                po = a * HD
                                nc.tensor.matmul(
                                    sp2[:, a * 512:(a + 1) * 512],
                                    lhsT=kT_sb[hp][po:po + HD,
                                                   kc * P:(kc + 1) * P],
                                    rhs=qT_sb[hp][po:po + HD, qs],
                                    start=True,
                                    stop=True,
                                )
                            em2 = empool.tile(
                                [P, 1024], bf16, tag="em",
                                name=f"em{hp}_{t}_{kc}"
                            )
                            nc.scalar.activation(em2, sp2, Exp, scale=0.125)
                            mb = (maskT_sb[:, kc * NQ + t * 512:
                                           kc * NQ + (t + 1) * 512]
                                  .rearrange("p (o q) -> p o q", o=1)
                                  .broadcast_to([P, 2, 512]))
                            em3 = em2.rearrange("p (o q) -> p o q", o=2)
                            nc.vector.tensor_mul(em3, em3, mb)
                            ems[kc] = em2
                        if it >= 1:
                            kd = it - 1
                            emd = ems.pop(kd)
                            # attnV: full-array M=65 per head (row 64 = the
                            # softmax denominator via the v ones-column)
                            for a, h in enumerate((h0, h1)):
                                nc.tensor.matmul(
                                    accs[a],
                                    lhsT=v_sb[kd][:, h * 65:(h + 1) * 65],
                                    rhs=emd[:, a * 512:(a + 1) * 512],
                                    start=(kd == 0),
                                    stop=(kd == KC - 1),
                                )
                        if it < KC:
                            n_emit = -(-len(q) // slots_left)  # ceil
                            for _ in range(min(n_emit, len(q))):
                                q.popleft()()
                            slots_left -= 1
                            if it == 2 and norm_a:
                                norm_a.popleft()()
                            if it == 6 and norm_b:
                                norm_b.popleft()()
                            if hp == 3 and t == 1 and it >= 8:
                                for _ in range(4):
                                    if wo_first:
                                        wo_first.popleft()()
                    # block epilogue: drain the acc banks fast (ut halves +
                    # reciprocal of the denominator rows), defer the rest
                    ut = work.tile([P, 512], f32, tag="ut",
                                   name=f"ut{hp}_{t}", bufs=2)
                    rs = []
                    for a in range(2):
                        nc.vector.tensor_copy(
                            ut[a * HD:(a + 1) * HD, :], accs[a][0:HD, :])
                        sf = work.tile([1, 512], f32, tag=f"s{a}",
                                       name=f"s{a}_{hp}_{t}", bufs=2)
                        nc.vector.tensor_copy(sf, accs[a][64:65, :])
                        r = work.tile([1, 512], f32, tag=f"r{a}",
                                      name=f"r{a}_{hp}_{t}", bufs=2)
                        nc.vector.reciprocal_approx_fast(r, sf)
                        rs.append(r)
                    na, nb = make_norm(hp, t, ut, rs[0], rs[1])
                    norm_a.append(na)
                    norm_b.append(nb)

            while wo_first:
                wo_first.popleft()()

            # ---- output projection, second q-half ----
            ops2 = wo_ops(range(4, NQ // P),
                          lambda t2, n: ((pmm, "mm") if (t2 * 2 + n) % 2 == 0
                                         else (psc, "sc")),
                          copy_eng="vector")
            # sc-tagged chains' first 3 MMs (c=0..2 don't read vals[3]) run
            # ahead of the final norm; their c=3 MM and the rest follow
            for j in (1, 3):
                for c in range(3):
                    ops2[j * MT + c]()
            while norm_a:
                norm_a.popleft()()
            while norm_b:
                norm_b.popleft()()
            for j in (1, 3):
                ops2[j * MT + 3]()
            for j in (0, 2):
                for c in range(MT):
                    ops2[j * MT + c]()
            for i in range(4 * MT, len(ops2)):
                ops2[i]()

    nc.compile()
    return nc


def _get_program():
    if "nc" not in _CACHE:
        _CACHE["nc"] = _build_program()
    return _CACHE["nc"]


def _swizzle_rows(a):
    """[D, C] -> [128, (D//128)*C]: chunk d's rows at free-offset d*C."""
    d128 = a.shape[0] // P
    return np.ascontiguousarray(
        a.reshape(d128, P, -1).transpose(1, 0, 2).reshape(P, -1)
    )


def _per_core_inputs(x, y, mask, W_kv, b_kv, W_q, b_q, W_o):
    """Build the 8 per-core input maps (host-preswizzled to SBUF layouts)."""
    in_maps = []
    mask_f = mask.astype(np.float32)
    for c in range(8):
        b, g = c // 2, c % 2
        gh = np.arange(g * 8, g * 8 + 8)
        k_cols = (gh[:, None] * 2 * HD + np.arange(HD)[None, :]).ravel()
        v_cols = k_cols + HD
        q_cols = slice(g * 512, (g + 1) * 512)

        xT = x[b].T                                  # [D, NKV]
        # xT dram [512, 4096]: row qb*128+p, col d*512+cc =
        #   xT[d*128+p, qb*512+cc]
        xr = xT.reshape(DC, P, 4, 512)               # [d, p, qb, cc]
        x_host = np.ascontiguousarray(
            xr.transpose(2, 1, 0, 3).reshape(512, 4096)).astype(BF)
        yT = y[b].T                                  # [D, NQ]
        y_host = _swizzle_rows(yT).astype(BF)        # [128, 8*1024]
        mT = mask_f[b].T                             # [NKV, NQ]
        m_host = _swizzle_rows(mT).astype(BF)        # [128, 16*1024]
        wk_host = _swizzle_rows(W_kv[:, k_cols]).astype(BF)
        wv_host = _swizzle_rows(W_kv[:, v_cols]).astype(BF)
        wq_host = _swizzle_rows(W_q[:, q_cols]).astype(BF)
        wo_host = _swizzle_rows(W_o[q_cols, :]).astype(BF)
        bk_host = np.ascontiguousarray(
            b_kv[k_cols].astype(np.float32).reshape(MT, P).T)
        bq_host = np.ascontiguousarray(
            b_q[np.arange(g * 512, (g + 1) * 512)]
            .astype(np.float32).reshape(MT, P).T)
        in_maps.append({
            "xT": x_host, "yT": y_host, "maskT": m_host,
            "wk": wk_host, "wv": wv_host, "wq": wq_host, "wo": wo_host,
            "bk": bk_host, "bq": bq_host,
        })
    return in_maps


def kernel(x, y, mask, W_kv, b_kv, W_q, b_q, W_o, b_o):
    from concourse import bass_utils

    x = np.asarray(x, np.float32)
    y = np.asarray(y, np.float32)
    mask = np.asarray(mask)
    W_kv = np.asarray(W_kv, np.float32)
    b_kv = np.asarray(b_kv, np.float32)
    W_q = np.asarray(W_q, np.float32)
    b_q = np.asarray(b_q, np.float32)
    W_o = np.asarray(W_o, np.float32)
    b_o = np.asarray(b_o, np.float32)

    nc = _get_program()
    in_maps = _per_core_inputs(x, y, mask, W_kv, b_kv, W_q, b_q, W_o)
    res = bass_utils.run_bass_kernel_spmd(nc, in_maps, core_ids=list(range(8)))

    # b_v folds into a constant row: attn rows sum to 1, so each head adds
    # b_v_h @ W_o_h to every output row; b_o adds on top.
    v_cols_all = (np.arange(H)[:, None] * 2 * HD + HD
                  + np.arange(HD)[None, :]).ravel()
    const_row = b_kv[v_cols_all].astype(np.float32) @ W_o + b_o

    out = np.empty((B, NQ, D), np.float32)
    for b in range(B):
        out[b] = (res.results[2 * b]["out"].astype(np.float32)
                  + res.results[2 * b + 1]["out"].astype(np.float32)
                  + const_row)
    return out


if __name__ == "__main__":
    import reference

    inputs = {k: np.asarray(v) for k, v in reference.setup_inputs().items()}
    got = kernel(**inputs)
    exp = np.asarray(reference.reference(**inputs))
    err = np.abs(got - exp)
    print("absmax rel err:", err.max() / np.abs(exp).max())


# revision 34
# speedup vs baseline: 1.1407x; 1.0018x over previous
"""Multi-head cross-attention Trainium2 Bass kernel, SPMD over 8 NeuronCores.

Sharding: core c handles batch b = c//2 and head group g = c%2 (8 of 16 heads).
Each core computes a partial output projection (its heads' W_o rows); the host
sums the two partials per batch element.

Device pipeline per core (all matmuls bf16 with fp32 PSUM accumulation):
  kT = (Wk^T x^T)          [512 hd, 2048 kseq]   (per-partition bias b_k)
  v  = (x Wv)              [2048 kseq, 8*64]
  qT = (Wq^T y^T)          [512 hd, 1024 q]      (per-partition bias b_q)
  per (head-pair, q-tile, k-chunk):
      S^T[k, q|q'] = kT_h^T-chunk @ qT_h for both heads of the pair
        (K=64 row-tiled at partitions 0/64 -> concurrent in the PE array)
      em = exp(0.125 * S^T)  (one ACT op per pair)
      em *= maskT            (one DVE mul per pair, head-broadcast)
      attnV: col-tiled pair into ONE psum bank: h0 -> vacc[0:64],
        h1 -> vacc[64:128]  (tile_position (0,0)/(0,64), concurrent)
      denominators: every 2nd kc, a 4-way col-tiled batch of ones-matmuls
        (strips at psum partitions 0/32/64/96) accumulates sum_k em for
        (even-kc h0, even h1, odd h0, odd h1) -> 2 kc amortized per 512 cyc
  normalize per block: dsum rows -> reciprocal_approx_fast -> bf16 ->
      one K=2 selector matmul (E2) broadcasts [2,512] over the 128
      partitions -> one [128,512] DVE mul into vals
  out_partial = vals^T-chunks @ Wo-rows  -> [1024 q, 1024 D]
Load phase: inputs are host-preswizzled to SBUF layout so each tensor is one
large contiguous DMA (the per-DMA issue cost on the Sync queue is ~0.6us;
many small DMAs made the old load phase issue-bound).  ~30 junk matmuls on a
zeroed tile warm the PE clock (HAM) during the DMA window.
b_v and b_o fold into a host-side constant row (attn rows sum to 1).
"""

import sys
from collections import deque

import numpy as np
import ml_dtypes

if "/opt/trn_rl_repo" not in sys.path:
    sys.path.insert(0, "/opt/trn_rl_repo")

BF = ml_dtypes.bfloat16

B, NKV, NQ, D, H = 4, 2048, 1024, 1024, 16
HD = D // H          # 64
NHL = 8              # heads per core (local)
P = 128
DC = D // P          # 8 contraction chunks over model dim
KC = NKV // P        # 16 key-seq chunks
QT = NQ // 512       # 2 q tiles of 512 for attention
MT = 4               # hd-dim chunks of kT/qT (512/128)

_CACHE = {}


def _build_program():
    import concourse.bass as bass
    import concourse.mybir as mybir
    import concourse.tile as tile
    from concourse import bacc

    f32 = mybir.dt.float32
    bf16 = mybir.dt.bfloat16

    nc = bacc.Bacc(
        "TRN2", target_bir_lowering=False, debug=False, num_devices=8
    )

    # host-preswizzled inputs: each is one contiguous DMA into its SBUF tile
    xT_d = nc.dram_tensor("xT", [512, 4096], bf16, kind="ExternalInput").ap()
    yT_d = nc.dram_tensor("yT", [P, DC * NQ], bf16, kind="ExternalInput").ap()
    maskT_d = nc.dram_tensor(
        "maskT", [P, KC * NQ], bf16, kind="ExternalInput"
    ).ap()
    wk_d = nc.dram_tensor("wk", [P, DC * 512], bf16, kind="ExternalInput").ap()
    wv_d = nc.dram_tensor("wv", [P, DC * 512], bf16, kind="ExternalInput").ap()
    wq_d = nc.dram_tensor("wq", [P, DC * 512], bf16, kind="ExternalInput").ap()
    wo_d = nc.dram_tensor("wo", [P, MT * D], bf16, kind="ExternalInput").ap()
    bk_d = nc.dram_tensor("bk", [P, MT], f32, kind="ExternalInput").ap()
    bq_d = nc.dram_tensor("bq", [P, MT], f32, kind="ExternalInput").ap()
    out_d = nc.dram_tensor("out", [NQ, D], bf16, kind="ExternalOutput").ap()

    Exp = mybir.ActivationFunctionType.Exp

    with tile.TileContext(nc) as tc:
        with (
            tc.tile_pool(name="persist", bufs=1) as persist,
            tc.tile_pool(name="work", bufs=3) as work,
            tc.tile_pool(name="empool", bufs=6) as empool,
            tc.tile_pool(name="pmm", bufs=2, space="PSUM") as pmm,
            tc.tile_pool(name="pacc", bufs=2, space="PSUM") as pacc,
            tc.tile_pool(name="psc", bufs=2, space="PSUM") as psc,
        ):
            # ---- persistent SBUF tiles ----
            wk_sb = persist.tile([P, DC * 512], bf16, tag="wk", name="wk")
            wv_sb = persist.tile([P, DC * 512], bf16, tag="wv", name="wv")
            wq_sb = persist.tile([P, DC * 512], bf16, tag="wq", name="wq")
            wo_sb = persist.tile([P, MT * D], bf16, tag="wo", name="wo")
            xT_sb = persist.tile([P, DC * NKV], bf16, tag="xT", name="xT")
            yT_sb = persist.tile([P, DC * NQ], bf16, tag="yT", name="yT")
            maskT_sb = persist.tile(
                [P, KC * NQ], bf16, tag="mT", name="mT"
            )
            bk_sb = persist.tile([P, MT], f32, tag="bk", name="bk")
            bq_sb = persist.tile([P, MT], f32, tag="bq", name="bq")

            def xs(d, lo, hi):     # xT chunk d, key-cols lo:hi
                return xT_sb[:, d * NKV + lo:d * NKV + hi]

            def ys(d, lo, hi):
                return yT_sb[:, d * NQ + lo:d * NQ + hi]

            # warm tile + constants
            warm = persist.tile([P, 256], bf16, tag="warm", name="warm")
            nc.gpsimd.memset(warm, 0.0)

            ones_row = persist.tile([1, HD], bf16, tag="onesr", name="onesr")
            nc.gpsimd.memset(ones_row, 1.0)

            # ---- HAM warmup: junk matmuls on the zeroed tile ----
            wps = psc.tile([P, 1024], f32, tag="sc", name="warmps")
            for i in range(64):
                nc.tensor.matmul(
                    wps[:, 0:256], lhsT=warm[:, 0:P], rhs=warm,
                    start=True, stop=True,
                )

            # ---- input DMAs: one per tensor (xT by column-quarters) ----
            nc.sync.dma_start(wk_sb, wk_d)
            nc.sync.dma_start(bk_sb, bk_d)
            nc.sync.dma_start(bq_sb, bq_d)
            x3 = xT_sb.rearrange("p (d c) -> p d c", d=DC)

            def load_xq(qb):
                nc.sync.dma_start(
                    x3[:, :, qb * 512:(qb + 1) * 512],
                    xT_d[qb * P:(qb + 1) * P, :].rearrange(
                        "p (d c) -> p d c", d=DC),
                )
            # first quarter in d-halves so the kT chain starts sooner
            nc.sync.dma_start(
                x3[:, 0:4, 0:512],
                xT_d[0:P, 0:4 * 512].rearrange("p (d c) -> p d c", d=4),
            )
            nc.sync.dma_start(
                x3[:, 4:8, 0:512],
                xT_d[0:P, 4 * 512:8 * 512].rearrange("p (d c) -> p d c", d=4),
            )
            nc.sync.dma_start(wv_sb, wv_d)
            for qb in range(1, 4):
                load_xq(qb)
            nc.sync.dma_start(wq_sb, wq_d)
            nc.sync.dma_start(yT_sb, yT_d)
            for mq in range(4):
                nc.sync.dma_start(
                    maskT_sb[:, mq * 4096:(mq + 1) * 4096],
                    maskT_d[:, mq * 4096:(mq + 1) * 4096],
                )
            nc.sync.dma_start(wo_sb, wo_d)

            kT_sb = [
                persist.tile([P, NKV], bf16, tag=f"kT{m}", name=f"kT{m}")
                for m in range(MT)
            ]
            qT_sb = [
                persist.tile([P, NQ], bf16, tag=f"qT{m}", name=f"qT{m}")
                for m in range(MT)
            ]
            v_sb = [
                persist.tile([P, NHL * 65], bf16, tag=f"v{i}", name=f"v{i}")
                for i in range(KC)
            ]
            for i in range(KC):
                nc.gpsimd.memset(
                    v_sb[i].rearrange("p (h c) -> p h c", c=65)[:, :, 64:65],
                    1.0,
                )
            vals_sb = [
                persist.tile([P, NQ], bf16, tag=f"vals{c}", name=f"vals{c}")
                for c in range(MT)
            ]

            # ---- projection op queues (closures; chain tail adds bias) ----
            def proj_ops(m, which, ns=None, pool=None):
                w_sb, dst, bias, ncols = (
                    (wk_sb, kT_sb, bk_sb, NKV) if which == "k"
                    else (wq_sb, qT_sb, bq_sb, NQ)
                )
                src = xs if which == "k" else ys
                pool_ = pool if pool is not None else pmm
                tag = "sc" if pool_ is psc else "mm"
                ops = []
                hold = {}
                nlist = list(range(ncols // 512) if ns is None else ns)
                for n in nlist:
                    for d in range(DC):
                        def op(m=m, n=n, d=d, w_sb=w_sb, dst=dst, bias=bias,
                               which=which, src=src, pool_=pool_, tag=tag):
                            if d == 0:
                                hold[n] = pool_.tile(
                                    [P, 512] if tag == "mm" else [P, 1024],
                                    f32, tag=tag, name=f"pj{which}{m}_{n}"
                                )
                            ps = hold[n][:, 0:512]
                            nc.tensor.matmul(
                                ps,
                                lhsT=w_sb[:, d * 512 + m * P:
                                          d * 512 + (m + 1) * P],
                                rhs=src(d, n * 512, (n + 1) * 512),
                                start=(d == 0),
                                stop=(d == DC - 1),
                            )
                            if d == DC - 1:
                                nc.vector.tensor_scalar_add(
                                    dst[m][:, n * 512:(n + 1) * 512],
                                    ps, bias[:, m:m + 1]
                                )
                        ops.append(op)
                return ops

            def v_ops(i, pool_):
                tag = "sc" if pool_ is psc else "mm"
                ops = []
                hold = {}

                def mk(d, i=i, pool_=pool_, tag=tag):
                    def op(d=d):
                        if d == 0:
                            hold[0] = pool_.tile(
                                [P, 512] if tag == "mm" else [P, 1024],
                                f32, tag=tag, name=f"ps_v{i}"
                            )
                        nc.tensor.matmul(
                            hold[0][:, 0:512],
                            lhsT=xs(d, i * P, (i + 1) * P),
                            rhs=wv_sb[:, d * 512:(d + 1) * 512],
                            start=(d == 0),
                            stop=(d == DC - 1),
                        )
                        if d == DC - 1:
                            v3 = v_sb[i].rearrange("p (h c) -> p h c", c=65)
                            nc.vector.tensor_copy(
                                v3[:, :, 0:64],
                                hold[0][:, 0:512].rearrange(
                                    "p (h c) -> p h c", c=64),
                            )
                    return op
                for d in range(DC):
                    ops.append(mk(d))
                return ops

            # upfront: kT chunk-0 chains as their xT quarters land, early v
            # chains, then qT chunk-0 — attention starts ASAP so ACT/DVE ramp
            for op in proj_ops(0, "k", ns=[0], pool=psc):
                op()
            for i in (0, 1, 2, 3):
                for op in v_ops(i, psc):
                    op()
            for op in proj_ops(0, "k", ns=[1], pool=psc):
                op()
            for i in (4, 5, 6, 7):
                for op in v_ops(i, psc):
                    op()
            for op in proj_ops(0, "k", ns=[2], pool=psc):
                op()
            for i in (8, 9):
                for op in v_ops(i, psc):
                    op()
            for op in proj_ops(0, "k", ns=[3], pool=psc):
                op()
            for i in (10, 11):
                for op in v_ops(i, psc):
                    op()
            for op in proj_ops(0, "q", pool=psc):
                op()

            # deferred work, drained inside the attention loops:
            # hp0 gets the v tail + chunk-1 projections, etc.
            pending = {
                0: deque(
                    v_ops(12, pmm) + v_ops(13, pmm) + v_ops(14, pmm)
                    + v_ops(15, pmm)
                    + proj_ops(1, "k") + proj_ops(1, "q")
                ),
                1: deque(proj_ops(2, "k") + proj_ops(2, "q")),
                2: deque(proj_ops(3, "k") + proj_ops(3, "q")),
                3: deque(),
            }

            # ---- output-projection chains (closures) ----
            def wo_ops(t2s, pool_pick, copy_eng="scalar", dma_eng=None):
                ops = []
                hold = {}
                for t2 in t2s:
                    for n in range(D // 512):
                        for c in range(MT):
                            def op(t2=t2, n=n, c=c):
                                if c == 0:
                                    pool, tag = pool_pick(t2, n)
                                    hold[(t2, n)] = pool.tile(
                                        [P, 512] if tag == "mm"
                                        else [P, 1024],
                                        f32, tag=tag, name=f"ps_o{t2}_{n}"
                                    )
                                ps_o = hold[(t2, n)][:, 0:512]
                                nc.tensor.matmul(
                                    ps_o,
                                    lhsT=vals_sb[c][:, t2 * P:(t2 + 1) * P],
                                    rhs=wo_sb[:, c * D + n * 512:
                                              c * D + (n + 1) * 512],
                                    start=(c == 0),
                                    stop=(c == MT - 1),
                                )
                                if c == MT - 1:
                                    ot = work.tile(
                                        [P, 512], bf16, tag="ot",
                                        name=f"ot{t2}_{n}", bufs=3
                                    )
                                    if copy_eng == "scalar":
                                        nc.scalar.copy(ot, ps_o)
                                    else:
                                        nc.vector.tensor_copy(ot, ps_o)
                                    eng = (nc.scalar if dma_eng == "scalar"
                                           and (t2 + n) % 2 == 0
                                           else nc.sync)
                                    eng.dma_start(
                                        out_d[t2 * P:(t2 + 1) * P,
                                              n * 512:(n + 1) * 512], ot
                                    )
                            ops.append(op)
                return ops

            wo_first = deque(wo_ops(range(0, 4), lambda t2, n: (pmm, "mm")))

            # ---- attention ----
            norm_a = deque()
            norm_b = deque()

            def make_norm(hp, t, ut, sf0, sf1):
                qs = slice(t * 512, (t + 1) * 512)

                def na():
                    rbs = []
                    for h, sf in ((0, sf0), (1, sf1)):
                        r = work.tile([1, 512], f32, tag=f"r{h}",
                                      name=f"r{h}_{hp}_{t}", bufs=2)
                        nc.vector.reciprocal_approx_fast(r, sf)
                        rb = work.tile([1, 512], bf16, tag=f"rb{h}",
                                       name=f"rb{h}_{hp}_{t}", bufs=2)
                        nc.vector.tensor_copy(rb, r)
                        rbs.append(rb)
                    na.rbs = rbs

                def nb():
                    bps = pmm.tile([P, 512], f32, tag="mm",
                                   name=f"bps{hp}_{t}")
                    for a in range(2):
                        nc.tensor.matmul(
                            bps[a * HD:(a + 1) * HD, :],
                            lhsT=ones_row, rhs=na.rbs[a],
                            start=True, stop=True,
                        )
                    nc.vector.tensor_mul(vals_sb[hp][:, qs], ut, bps)
                return na, nb

            for hp in range(MT):
                h0, h1 = 2 * hp, 2 * hp + 1
                q = pending[hp]
                for t in range(QT):
                    qs = slice(t * 512, (t + 1) * 512)
                    slots_left = (QT - t) * KC
                    accs = [
                        pacc.tile([65, 512], f32, tag="acc",
                                  name=f"acc{h}_{t}")
                        for h in (h0, h1)
                    ]
                    ems = {}
                    # software-pipelined: attnV/denom for kc-1 are emitted
                    # one iteration behind scores/exp/mask(kc), so the PE
                    # never sits on the exp->mask latency; pending drains
                    # go AFTER the attention ops so mask keeps DVE priority
                    for it in range(KC + 1):
                        if it < KC:
                            kc = it
                            sp2 = psc.tile(
                                [P, 1024], f32, tag="sc",
                                name=f"sp{hp}_{t}_{kc}"
                            )
                            for a in range(2):
                                po = a * HD
                                nc.tensor.matmul(
                                    sp2[:, a * 512:(a + 1) * 512],
                                    lhsT=kT_sb[hp][po:po + HD,
                                                   kc * P:(kc + 1) * P],
                                    rhs=qT_sb[hp][po:po + HD, qs],
                                    start=True,
                                    stop=True,
                                )
                            em2 = empool.tile(
                                [P, 1024], bf16, tag="em",
                                name=f"em{hp}_{t}_{kc}"
                            )
                            nc.scalar.activation(em2, sp2, Exp, scale=0.125)
                            mb = (maskT_sb[:, kc * NQ + t * 512:
                                           kc * NQ + (t + 1) * 512]
                                  .rearrange("p (o q) -> p o q", o=1)
                                  .broadcast_to([P, 2, 512]))
                            em3 = em2.rearrange("p (o q) -> p o q", o=2)
                            nc.vector.tensor_mul(em3, em3, mb)
                            ems[kc] = em2
                        if it >= 1:
                            kd = it - 1
                            emd = ems.pop(kd)
                            # attnV: full-array M=65 per head (row 64 = the
                            # softmax denominator via the v ones-column)
                            for a, h in enumerate((h0, h1)):
                                nc.tensor.matmul(
                                    accs[a],
                                    lhsT=v_sb[kd][:, h * 65:(h + 1) * 65],
                                    rhs=emd[:, a * 512:(a + 1) * 512],
                                    start=(kd == 0),
                                    stop=(kd == KC - 1),
                                )
                        if it < KC:
                            n_emit = -(-len(q) // slots_left)  # ceil
                            for _ in range(min(n_emit, len(q))):
                                q.popleft()()
                            slots_left -= 1
                            if it == 2 and norm_a:
                                norm_a.popleft()()
                            if it == 6 and norm_b:
                                norm_b.popleft()()
                            if hp == 3 and t == 1 and it >= 8:
                                for _ in range(4):
                                    if wo_first:
                                        wo_first.popleft()()
                    # block epilogue: drain the acc banks fast (ut halves +
                    # reciprocal of the denominator rows), defer the rest
                    ut = work.tile([P, 512], f32, tag="ut",
                                   name=f"ut{hp}_{t}", bufs=2)
                    sfs = []
                    for a in range(2):
                        nc.vector.tensor_copy(
                            ut[a * HD:(a + 1) * HD, :], accs[a][0:HD, :])
                        sf = work.tile([1, 512], f32, tag=f"s{a}",
                                       name=f"s{a}_{hp}_{t}", bufs=2)
                        nc.scalar.copy(sf, accs[a][64:65, :])
                        sfs.append(sf)
                    na, nb = make_norm(hp, t, ut, sfs[0], sfs[1])
                    norm_a.append(na)
                    norm_b.append(nb)

            while wo_first:
                wo_first.popleft()()

            # ---- output projection, second q-half ----
            ops2 = wo_ops(range(4, NQ // P),
                          lambda t2, n: ((pmm, "mm") if (t2 * 2 + n) % 2 == 0
                                         else (psc, "sc")),
                          copy_eng="vector", dma_eng="scalar")
            # sc-tagged chains' first 3 MMs (c=0..2 don't read vals[3]) run
            # ahead of the final norm; their c=3 MM and the rest follow
            for j in (1, 3):
                for c in range(3):
                    ops2[j * MT + c]()
            while norm_a:
                norm_a.popleft()()
            while norm_b:
                norm_b.popleft()()
            for j in (1, 3):
                ops2[j * MT + 3]()
            for j in (0, 2):
                for c in range(MT):
                    ops2[j * MT + c]()
            for i in range(4 * MT, len(ops2)):
                ops2[i]()

    nc.compile()
    return nc


def _get_program():
    if "nc" not in _CACHE:
        _CACHE["nc"] = _build_program()
    return _CACHE["nc"]


def _swizzle_rows(a):
    """[D, C] -> [128, (D//128)*C]: chunk d's rows at free-offset d*C."""
    d128 = a.shape[0] // P
    return np.ascontiguousarray(
        a.reshape(d128, P, -1).transpose(1, 0, 2).reshape(P, -1)
    )


def _per_core_inputs(x, y, mask, W_kv, b_kv, W_q, b_q, W_o):
    """Build the 8 per-core input maps (host-preswizzled to SBUF layouts)."""
    in_maps = []
    mask_f = mask.astype(np.float32)
    for c in range(8):
        b, g = c // 2, c % 2
        gh = np.arange(g * 8, g * 8 + 8)
        k_cols = (gh[:, None] * 2 * HD + np.arange(HD)[None, :]).ravel()
        v_cols = k_cols + HD
        q_cols = slice(g * 512, (g + 1) * 512)

        xT = x[b].T                                  # [D, NKV]
        # xT dram [512, 4096]: row qb*128+p, col d*512+cc =
        #   xT[d*128+p, qb*512+cc]
        xr = xT.reshape(DC, P, 4, 512)               # [d, p, qb, cc]
        x_host = np.ascontiguousarray(
            xr.transpose(2, 1, 0, 3).reshape(512, 4096)).astype(BF)
        yT = y[b].T                                  # [D, NQ]
        y_host = _swizzle_rows(yT).astype(BF)        # [128, 8*1024]
        mT = mask_f[b].T                             # [NKV, NQ]
        m_host = _swizzle_rows(mT).astype(BF)        # [128, 16*1024]
        wk_host = _swizzle_rows(W_kv[:, k_cols]).astype(BF)
        wv_host = _swizzle_rows(W_kv[:, v_cols]).astype(BF)
        wq_host = _swizzle_rows(W_q[:, q_cols]).astype(BF)
        wo_host = _swizzle_rows(W_o[q_cols, :]).astype(BF)
        bk_host = np.ascontiguousarray(
            b_kv[k_cols].astype(np.float32).reshape(MT, P).T)
        bq_host = np.ascontiguousarray(
            b_q[np.arange(g * 512, (g + 1) * 512)]
            .astype(np.float32).reshape(MT, P).T)
        in_maps.append({
            "xT": x_host, "yT": y_host, "maskT": m_host,
            "wk": wk_host, "wv": wv_host, "wq": wq_host, "wo": wo_host,
            "bk": bk_host, "bq": bq_host,
        })
    return in_maps


def kernel(x, y, mask, W_kv, b_kv, W_q, b_q, W_o, b_o):
    from concourse import bass_utils

    x = np.asarray(x, np.float32)
    y = np.asarray(y, np.float32)
    mask = np.asarray(mask)
    W_kv = np.asarray(W_kv, np.float32)
    b_kv = np.asarray(b_kv, np.float32)
    W_q = np.asarray(W_q, np.float32)
    b_q = np.asarray(b_q, np.float32)
    W_o = np.asarray(W_o, np.float32)
    b_o = np.asarray(b_o, np.float32)

    nc = _get_program()
    in_maps = _per_core_inputs(x, y, mask, W_kv, b_kv, W_q, b_q, W_o)
    res = bass_utils.run_bass_kernel_spmd(nc, in_maps, core_ids=list(range(8)))

    # b_v folds into a constant row: attn rows sum to 1, so each head adds
    # b_v_h @ W_o_h to every output row; b_o adds on top.
    v_cols_all = (np.arange(H)[:, None] * 2 * HD + HD
                  + np.arange(HD)[None, :]).ravel()
    const_row = b_kv[v_cols_all].astype(np.float32) @ W_o + b_o

    out = np.empty((B, NQ, D), np.float32)
    for b in range(B):
        out[b] = (res.results[2 * b]["out"].astype(np.float32)
                  + res.results[2 * b + 1]["out"].astype(np.float32)
                  + const_row)
    return out


if __name__ == "__main__":
    import reference

    inputs = {k: np.asarray(v) for k, v in reference.setup_inputs().items()}
    got = kernel(**inputs)
    exp = np.asarray(reference.reference(**inputs))
    err = np.abs(got - exp)
    print("absmax rel err:", err.max() / np.abs(exp).max())


# revision 35
# speedup vs baseline: 1.1644x; 1.0208x over previous
"""Multi-head cross-attention Trainium2 Bass kernel, SPMD over 8 NeuronCores.

Sharding: core c handles batch b = c//2 and head group g = c%2 (8 of 16 heads).
Each core computes a partial output projection (its heads' W_o rows); the host
sums the two partials per batch element.

Device pipeline per core (all matmuls bf16 with fp32 PSUM accumulation):
  kT = (Wk^T x^T)          [512 hd, 2048 kseq]   (per-partition bias b_k)
  v  = (x Wv)              [2048 kseq, 8*64]
  qT = (Wq^T y^T)          [512 hd, 1024 q]      (per-partition bias b_q)
  per (head-pair, q-tile, k-chunk):
      S^T[k, q|q'] = kT_h^T-chunk @ qT_h for both heads of the pair
        (K=64 row-tiled at partitions 0/64 -> concurrent in the PE array)
      em = exp(0.125 * S^T)  (one ACT op per pair)
      em *= maskT            (one DVE mul per pair, head-broadcast)
      attnV: col-tiled pair into ONE psum bank: h0 -> vacc[0:64],
        h1 -> vacc[64:128]  (tile_position (0,0)/(0,64), concurrent)
      denominators: every 2nd kc, a 4-way col-tiled batch of ones-matmuls
        (strips at psum partitions 0/32/64/96) accumulates sum_k em for
        (even-kc h0, even h1, odd h0, odd h1) -> 2 kc amortized per 512 cyc
  normalize per block: dsum rows -> reciprocal_approx_fast -> bf16 ->
      one K=2 selector matmul (E2) broadcasts [2,512] over the 128
      partitions -> one [128,512] DVE mul into vals
  out_partial = vals^T-chunks @ Wo-rows  -> [1024 q, 1024 D]
Load phase: inputs are host-preswizzled to SBUF layout so each tensor is one
large contiguous DMA (the per-DMA issue cost on the Sync queue is ~0.6us;
many small DMAs made the old load phase issue-bound).  ~30 junk matmuls on a
zeroed tile warm the PE clock (HAM) during the DMA window.
b_v and b_o fold into a host-side constant row (attn rows sum to 1).
"""

import sys
from collections import deque

import numpy as np
import ml_dtypes

if "/opt/trn_rl_repo" not in sys.path:
    sys.path.insert(0, "/opt/trn_rl_repo")

BF = ml_dtypes.bfloat16

B, NKV, NQ, D, H = 4, 2048, 1024, 1024, 16
HD = D // H          # 64
NHL = 8              # heads per core (local)
P = 128
DC = D // P          # 8 contraction chunks over model dim
KC = NKV // P        # 16 key-seq chunks
QT = NQ // 512       # 2 q tiles of 512 for attention
MT = 4               # hd-dim chunks of kT/qT (512/128)

_CACHE = {}


def _build_program():
    import concourse.bass as bass
    import concourse.mybir as mybir
    import concourse.tile as tile
    from concourse import bacc

    f32 = mybir.dt.float32
    bf16 = mybir.dt.bfloat16

    nc = bacc.Bacc(
        "TRN2", target_bir_lowering=False, debug=False, num_devices=8
    )

    # host-preswizzled inputs: each is one contiguous DMA into its SBUF tile
    xT_d = nc.dram_tensor("xT", [512, 4096], bf16, kind="ExternalInput").ap()
    yT_d = nc.dram_tensor("yT", [P, DC * NQ], bf16, kind="ExternalInput").ap()
    maskT_d = nc.dram_tensor(
        "maskT", [P, KC * NQ], bf16, kind="ExternalInput"
    ).ap()
    wk_d = nc.dram_tensor("wk", [P, DC * 512], bf16, kind="ExternalInput").ap()
    wv_d = nc.dram_tensor("wv", [P, DC * 512], bf16, kind="ExternalInput").ap()
    wq_d = nc.dram_tensor("wq", [P, DC * 512], bf16, kind="ExternalInput").ap()
    wo_d = nc.dram_tensor("wo", [P, MT * D], bf16, kind="ExternalInput").ap()
    bk_d = nc.dram_tensor("bk", [P, MT], f32, kind="ExternalInput").ap()
    bq_d = nc.dram_tensor("bq", [P, MT], f32, kind="ExternalInput").ap()
    out_d = nc.dram_tensor("out", [NQ, D], bf16, kind="ExternalOutput").ap()

    Exp = mybir.ActivationFunctionType.Exp

    with tile.TileContext(nc) as tc:
        with (
            tc.tile_pool(name="persist", bufs=1) as persist,
            tc.tile_pool(name="work", bufs=3) as work,
            tc.tile_pool(name="empool", bufs=6) as empool,
            tc.tile_pool(name="pmm", bufs=2, space="PSUM") as pmm,
            tc.tile_pool(name="pacc", bufs=2, space="PSUM") as pacc,
            tc.tile_pool(name="psc", bufs=2, space="PSUM") as psc,
        ):
            # ---- persistent SBUF tiles ----
            wk_sb = persist.tile([P, DC * 512], bf16, tag="wk", name="wk")
            wv_sb = persist.tile([P, DC * 512], bf16, tag="wv", name="wv")
            wq_sb = persist.tile([P, DC * 512], bf16, tag="wq", name="wq")
            wo_sb = persist.tile([P, MT * D], bf16, tag="wo", name="wo")
            xT_sb = persist.tile([P, DC * NKV], bf16, tag="xT", name="xT")
            yT_sb = persist.tile([P, DC * NQ], bf16, tag="yT", name="yT")
            maskT_sb = persist.tile(
                [P, KC * NQ], bf16, tag="mT", name="mT"
            )
            bk_sb = persist.tile([P, MT], f32, tag="bk", name="bk")
            bq_sb = persist.tile([P, MT], f32, tag="bq", name="bq")

            def xs(d, lo, hi):     # xT chunk d, key-cols lo:hi
                return xT_sb[:, d * NKV + lo:d * NKV + hi]

            def ys(d, lo, hi):
                return yT_sb[:, d * NQ + lo:d * NQ + hi]

            # warm tile + constants
            warm = persist.tile([P, 256], bf16, tag="warm", name="warm")
            nc.gpsimd.memset(warm, 0.0)

            ones_row = persist.tile([1, HD], bf16, tag="onesr", name="onesr")
            nc.gpsimd.memset(ones_row, 1.0)

            # ---- HAM warmup: junk matmuls on the zeroed tile ----
            wps = psc.tile([P, 1024], f32, tag="sc", name="warmps")
            for i in range(64):
                nc.tensor.matmul(
                    wps[:, 0:256], lhsT=warm[:, 0:P], rhs=warm,
                    start=True, stop=True,
                )

            # ---- input DMAs: one per tensor (xT by column-quarters) ----
            nc.sync.dma_start(wk_sb, wk_d)
            nc.sync.dma_start(bk_sb, bk_d)
            nc.sync.dma_start(bq_sb, bq_d)
            x3 = xT_sb.rearrange("p (d c) -> p d c", d=DC)

            def load_xq(qb):
                nc.sync.dma_start(
                    x3[:, :, qb * 512:(qb + 1) * 512],
                    xT_d[qb * P:(qb + 1) * P, :].rearrange(
                        "p (d c) -> p d c", d=DC),
                )
            # first quarter in d-halves so the kT chain starts sooner
            nc.sync.dma_start(
                x3[:, 0:4, 0:512],
                xT_d[0:P, 0:4 * 512].rearrange("p (d c) -> p d c", d=4),
            )
            nc.sync.dma_start(
                x3[:, 4:8, 0:512],
                xT_d[0:P, 4 * 512:8 * 512].rearrange("p (d c) -> p d c", d=4),
            )
            nc.sync.dma_start(wv_sb, wv_d)
            for qb in range(1, 4):
                load_xq(qb)
            nc.sync.dma_start(wq_sb, wq_d)
            nc.sync.dma_start(yT_sb, yT_d)
            for mq in range(4):
                nc.sync.dma_start(
                    maskT_sb[:, mq * 4096:(mq + 1) * 4096],
                    maskT_d[:, mq * 4096:(mq + 1) * 4096],
                )
            nc.sync.dma_start(wo_sb, wo_d)

            kT_sb = [
                persist.tile([P, NKV], bf16, tag=f"kT{m}", name=f"kT{m}")
                for m in range(MT)
            ]
            qT_sb = [
                persist.tile([P, NQ], bf16, tag=f"qT{m}", name=f"qT{m}")
                for m in range(MT)
            ]
            v_sb = [
                persist.tile([P, NHL * 65], bf16, tag=f"v{i}", name=f"v{i}")
                for i in range(KC)
            ]
            for i in range(KC):
                nc.gpsimd.memset(
                    v_sb[i].rearrange("p (h c) -> p h c", c=65)[:, :, 64:65],
                    1.0,
                )
            vals_sb = [
                persist.tile([P, NQ], bf16, tag=f"vals{c}", name=f"vals{c}")
                for c in range(MT)
            ]

            # ---- projection op queues (closures; chain tail adds bias) ----
            def proj_ops(m, which, ns=None, pool=None):
                w_sb, dst, bias, ncols = (
                    (wk_sb, kT_sb, bk_sb, NKV) if which == "k"
                    else (wq_sb, qT_sb, bq_sb, NQ)
                )
                src = xs if which == "k" else ys
                pool_ = pool if pool is not None else pmm
                tag = "sc" if pool_ is psc else "mm"
                ops = []
                hold = {}
                nlist = list(range(ncols // 512) if ns is None else ns)
                for n in nlist:
                    for d in range(DC):
                        def op(m=m, n=n, d=d, w_sb=w_sb, dst=dst, bias=bias,
                               which=which, src=src, pool_=pool_, tag=tag):
                            if d == 0:
                                hold[n] = pool_.tile(
                                    [P, 512] if tag == "mm" else [P, 1024],
                                    f32, tag=tag, name=f"pj{which}{m}_{n}"
                                )
                            ps = hold[n][:, 0:512]
                            nc.tensor.matmul(
                                ps,
                                lhsT=w_sb[:, d * 512 + m * P:
                                          d * 512 + (m + 1) * P],
                                rhs=src(d, n * 512, (n + 1) * 512),
                                start=(d == 0),
                                stop=(d == DC - 1),
                            )
                            if d == DC - 1:
                                nc.vector.tensor_scalar_add(
                                    dst[m][:, n * 512:(n + 1) * 512],
                                    ps, bias[:, m:m + 1]
                                )
                        ops.append(op)
                return ops

            def v_ops(i, pool_):
                tag = "sc" if pool_ is psc else "mm"
                ops = []
                hold = {}

                def mk(d, i=i, pool_=pool_, tag=tag):
                    def op(d=d):
                        if d == 0:
                            hold[0] = pool_.tile(
                                [P, 512] if tag == "mm" else [P, 1024],
                                f32, tag=tag, name=f"ps_v{i}"
                            )
                        nc.tensor.matmul(
                            hold[0][:, 0:512],
                            lhsT=xs(d, i * P, (i + 1) * P),
                            rhs=wv_sb[:, d * 512:(d + 1) * 512],
                            start=(d == 0),
                            stop=(d == DC - 1),
                        )
                        if d == DC - 1:
                            v3 = v_sb[i].rearrange("p (h c) -> p h c", c=65)
                            nc.vector.tensor_copy(
                                v3[:, :, 0:64],
                                hold[0][:, 0:512].rearrange(
                                    "p (h c) -> p h c", c=64),
                            )
                    return op
                for d in range(DC):
                    ops.append(mk(d))
                return ops

            # upfront: kT chunk-0 chains as their xT quarters land, early v
            # chains, then qT chunk-0 — attention starts ASAP so ACT/DVE ramp
            for op in proj_ops(0, "k", ns=[0], pool=psc):
                op()
            for i in (0, 1, 2, 3):
                for op in v_ops(i, psc):
                    op()
            for op in proj_ops(0, "k", ns=[1], pool=psc):
                op()
            for i in (4, 5, 6, 7):
                for op in v_ops(i, psc):
                    op()
            for op in proj_ops(0, "k", ns=[2], pool=psc):
                op()
            for i in (8, 9):
                for op in v_ops(i, psc):
                    op()
            for op in proj_ops(0, "k", ns=[3], pool=psc):
                op()
            for i in (10, 11):
                for op in v_ops(i, psc):
                    op()
            for op in proj_ops(0, "q", pool=psc):
                op()

            # deferred work, drained inside the attention loops:
            # hp0 gets the v tail + chunk-1 projections, etc.
            pending = {
                0: deque(
                    v_ops(12, pmm) + v_ops(13, pmm) + v_ops(14, pmm)
                    + v_ops(15, pmm)
                    + proj_ops(1, "k") + proj_ops(1, "q")
                ),
                1: deque(proj_ops(2, "k") + proj_ops(2, "q")),
                2: deque(proj_ops(3, "k") + proj_ops(3, "q")),
                3: deque(),
            }

            # ---- output-projection chains (closures) ----
            def wo_ops(t2s, pool_pick, copy_eng="scalar", dma_eng=None):
                ops = []
                hold = {}
                for t2 in t2s:
                    for n in range(D // 512):
                        for c in range(MT):
                            def op(t2=t2, n=n, c=c):
                                if c == 0:
                                    pool, tag = pool_pick(t2, n)
                                    hold[(t2, n)] = pool.tile(
                                        [P, 512] if tag == "mm"
                                        else [P, 1024],
                                        f32, tag=tag, name=f"ps_o{t2}_{n}"
                                    )
                                ps_o = hold[(t2, n)][:, 0:512]
                                nc.tensor.matmul(
                                    ps_o,
                                    lhsT=vals_sb[c][:, t2 * P:(t2 + 1) * P],
                                    rhs=wo_sb[:, c * D + n * 512:
                                              c * D + (n + 1) * 512],
                                    start=(c == 0),
                                    stop=(c == MT - 1),
                                )
                                if c == MT - 1:
                                    ot = work.tile(
                                        [P, 512], bf16, tag="ot",
                                        name=f"ot{t2}_{n}", bufs=3
                                    )
                                    if copy_eng == "scalar":
                                        nc.scalar.copy(ot, ps_o)
                                    else:
                                        nc.vector.tensor_copy(ot, ps_o)
                                    eng = (nc.scalar if dma_eng == "scalar"
                                           and (t2 + n) % 2 == 0
                                           else nc.sync)
                                    eng.dma_start(
                                        out_d[t2 * P:(t2 + 1) * P,
                                              n * 512:(n + 1) * 512], ot
                                    )
                            ops.append(op)
                return ops

            wo_first = deque(wo_ops(range(0, 4), lambda t2, n: (pmm, "mm")))

            # ---- attention ----
            norm_a = deque()
            norm_b = deque()

            def make_norm(hp, t, ut, sf0, sf1):
                qs = slice(t * 512, (t + 1) * 512)

                def na():
                    rbs = []
                    for h, sf in ((0, sf0), (1, sf1)):
                        r = work.tile([1, 512], f32, tag=f"r{h}",
                                      name=f"r{h}_{hp}_{t}", bufs=2)
                        nc.vector.reciprocal_approx_fast(r, sf)
                        rb = work.tile([1, 512], bf16, tag=f"rb{h}",
                                       name=f"rb{h}_{hp}_{t}", bufs=2)
                        nc.vector.tensor_copy(rb, r)
                        rbs.append(rb)
                    na.rbs = rbs

                def nb():
                    bps = pmm.tile([P, 512], f32, tag="mm",
                                   name=f"bps{hp}_{t}")
                    for a in range(2):
                        nc.tensor.matmul(
                            bps[a * HD:(a + 1) * HD, :],
                            lhsT=ones_row, rhs=na.rbs[a],
                            start=True, stop=True,
                        )
                    nc.vector.tensor_mul(vals_sb[hp][:, qs], ut, bps)
                return na, nb

            # Flat software pipeline over all 8 (hp, t) blocks x 16 kc:
            # iteration g emits scores/exp/mask for global step g and
            # attnV for step g-1 — continuous across block boundaries, so
            # neither ACT nor PE sees a bubble when a block ends.  The
            # epilogue of a finished block rides inside the next block's
            # first iteration (after its mask, so DVE priority holds).
            blocks = [(hp, t) for hp in range(MT) for t in range(QT)]
            NB = len(blocks)
            st = {}      # per-block state: accs, ems
            slots = {hp: 2 * KC for hp in range(MT)}

            def emit_front(g):
                bi, kc = divmod(g, KC)
                hp, t = blocks[bi]
                qs = slice(t * 512, (t + 1) * 512)
                if kc == 0:
                    st[bi] = {
                        "accs": [
                            pacc.tile([65, 512], f32, tag="acc",
                                      name=f"acc{bi}_{h}")
                            for h in (0, 1)
                        ],
                        "ems": {},
                    }
                sp2 = psc.tile(
                    [P, 1024], f32, tag="sc", name=f"sp{bi}_{kc}"
                )
                for a in range(2):
                    po = a * HD
                    nc.tensor.matmul(
                        sp2[:, a * 512:(a + 1) * 512],
                        lhsT=kT_sb[hp][po:po + HD, kc * P:(kc + 1) * P],
                        rhs=qT_sb[hp][po:po + HD, qs],
                        start=True,
                        stop=True,
                    )
                em2 = empool.tile(
                    [P, 1024], bf16, tag="em", name=f"em{bi}_{kc}"
                )
                nc.scalar.activation(em2, sp2, Exp, scale=0.125)
                mb = (maskT_sb[:, kc * NQ + t * 512:kc * NQ + (t + 1) * 512]
                      .rearrange("p (o q) -> p o q", o=1)
                      .broadcast_to([P, 2, 512]))
                em3 = em2.rearrange("p (o q) -> p o q", o=2)
                nc.vector.tensor_mul(em3, em3, mb)
                st[bi]["ems"][kc] = em2

            def emit_back(g):
                bj, kd = divmod(g, KC)
                hp, t = blocks[bj]
                s = st[bj]
                emd = s["ems"].pop(kd)
                # attnV: full-array M=65 per head (row 64 = the softmax
                # denominator via the v ones-column)
                for a, h in enumerate((2 * hp, 2 * hp + 1)):
                    nc.tensor.matmul(
                        s["accs"][a],
                        lhsT=v_sb[kd][:, h * 65:(h + 1) * 65],
                        rhs=emd[:, a * 512:(a + 1) * 512],
                        start=(kd == 0),
                        stop=(kd == KC - 1),
                    )
                if kd == KC - 1:
                    # epilogue: drain the acc banks (ut halves on DVE,
                    # denominator rows on ACT), defer recip/cast/bcast
                    ut = work.tile([P, 512], f32, tag="ut",
                                   name=f"ut{bj}", bufs=2)
                    sfs = []
                    for a in range(2):
                        nc.vector.tensor_copy(
                            ut[a * HD:(a + 1) * HD, :],
                            s["accs"][a][0:HD, :])
                        sf = work.tile([1, 512], f32, tag=f"s{a}",
                                       name=f"s{a}_{bj}", bufs=2)
                        nc.scalar.copy(sf, s["accs"][a][64:65, :])
                        sfs.append(sf)
                    na, nb = make_norm(hp, t, ut, sfs[0], sfs[1])
                    norm_a.append(na)
                    norm_b.append(nb)
                    del st[bj]

            for g in range(NB * KC + 1):
                if g < NB * KC:
                    emit_front(g)
                if g >= 1:
                    emit_back(g - 1)
                if g < NB * KC:
                    bi, kc = divmod(g, KC)
                    hp, t = blocks[bi]
                    q = pending[hp]
                    n_emit = -(-len(q) // slots[hp])  # ceil
                    for _ in range(min(n_emit, len(q))):
                        q.popleft()()
                    slots[hp] -= 1
                    last = (bi == NB - 1)
                    if kc == (1 if last else 2) and norm_a:
                        norm_a.popleft()()
                    if kc == (3 if last else 6) and norm_b:
                        norm_b.popleft()()
                    if last and kc >= 4:
                        for _ in range(3):
                            if wo_first:
                                wo_first.popleft()()

            while wo_first:
                wo_first.popleft()()

            # ---- output projection, second q-half ----
            ops2 = wo_ops(range(4, NQ // P),
                          lambda t2, n: ((pmm, "mm") if (t2 * 2 + n) % 2 == 0
                                         else (psc, "sc")),
                          copy_eng="vector", dma_eng="scalar")
            # sc-tagged chains' first 3 MMs (c=0..2 don't read vals[3]) run
            # ahead of the final norm; their c=3 MM and the rest follow
            for j in (1, 3):
                for c in range(3):
                    ops2[j * MT + c]()
            while norm_a:
                norm_a.popleft()()
            while norm_b:
                norm_b.popleft()()
            for j in (1, 3):
                ops2[j * MT + 3]()
            for j in (0, 2):
                for c in range(MT):
                    ops2[j * MT + c]()
            for i in range(4 * MT, len(ops2)):
                ops2[i]()

    nc.compile()
    return nc


def _get_program():
    if "nc" not in _CACHE:
        _CACHE["nc"] = _build_program()
    return _CACHE["nc"]


def _swizzle_rows(a):
    """[D, C] -> [128, (D//128)*C]: chunk d's rows at free-offset d*C."""
    d128 = a.shape[0] // P
    return np.ascontiguousarray(
        a.reshape(d128, P, -1).transpose(1, 0, 2).reshape(P, -1)
    )


def _per_core_inputs(x, y, mask, W_kv, b_kv, W_q, b_q, W_o):
    """Build the 8 per-core input maps (host-preswizzled to SBUF layouts)."""
    in_maps = []
    mask_f = mask.astype(np.float32)
    for c in range(8):
        b, g = c // 2, c % 2
        gh = np.arange(g * 8, g * 8 + 8)
        k_cols = (gh[:, None] * 2 * HD + np.arange(HD)[None, :]).ravel()
        v_cols = k_cols + HD
        q_cols = slice(g * 512, (g + 1) * 512)

        xT = x[b].T                                  # [D, NKV]
        # xT dram [512, 4096]: row qb*128+p, col d*512+cc =
        #   xT[d*128+p, qb*512+cc]
        xr = xT.reshape(DC, P, 4, 512)               # [d, p, qb, cc]
        x_host = np.ascontiguousarray(
            xr.transpose(2, 1, 0, 3).reshape(512, 4096)).astype(BF)
        yT = y[b].T                                  # [D, NQ]
        y_host = _swizzle_rows(yT).astype(BF)        # [128, 8*1024]
        mT = mask_f[b].T                             # [NKV, NQ]
        m_host = _swizzle_rows(mT).astype(BF)        # [128, 16*1024]
        wk_host = _swizzle_rows(W_kv[:, k_cols]).astype(BF)
        wv_host = _swizzle_rows(W_kv[:, v_cols]).astype(BF)
        wq_host = _swizzle_rows(W_q[:, q_cols]).astype(BF)
        wo_host = _swizzle_rows(W_o[q_cols, :]).astype(BF)
        bk_host = np.ascontiguousarray(
            b_kv[k_cols].astype(np.float32).reshape(MT, P).T)
        bq_host = np.ascontiguousarray(
            b_q[np.arange(g * 512, (g + 1) * 512)]
            .astype(np.float32).reshape(MT, P).T)
        in_maps.append({
            "xT": x_host, "yT": y_host, "maskT": m_host,
            "wk": wk_host, "wv": wv_host, "wq": wq_host, "wo": wo_host,
            "bk": bk_host, "bq": bq_host,
        })
    return in_maps


def kernel(x, y, mask, W_kv, b_kv, W_q, b_q, W_o, b_o):
    from concourse import bass_utils

    x = np.asarray(x, np.float32)
    y = np.asarray(y, np.float32)
    mask = np.asarray(mask)
    W_kv = np.asarray(W_kv, np.float32)
    b_kv = np.asarray(b_kv, np.float32)
    W_q = np.asarray(W_q, np.float32)
    b_q = np.asarray(b_q, np.float32)
    W_o = np.asarray(W_o, np.float32)
    b_o = np.asarray(b_o, np.float32)

    nc = _get_program()
    in_maps = _per_core_inputs(x, y, mask, W_kv, b_kv, W_q, b_q, W_o)
    res = bass_utils.run_bass_kernel_spmd(nc, in_maps, core_ids=list(range(8)))

    # b_v folds into a constant row: attn rows sum to 1, so each head adds
    # b_v_h @ W_o_h to every output row; b_o adds on top.
    v_cols_all = (np.arange(H)[:, None] * 2 * HD + HD
                  + np.arange(HD)[None, :]).ravel()
    const_row = b_kv[v_cols_all].astype(np.float32) @ W_o + b_o

    out = np.empty((B, NQ, D), np.float32)
    for b in range(B):
        out[b] = (res.results[2 * b]["out"].astype(np.float32)
                  + res.results[2 * b + 1]["out"].astype(np.float32)
                  + const_row)
    return out


if __name__ == "__main__":
    import reference

    inputs = {k: np.asarray(v) for k, v in reference.setup_inputs().items()}
    got = kernel(**inputs)
    exp = np.asarray(reference.reference(**inputs))
    err = np.abs(got - exp)
    print("absmax rel err:", err.max() / np.abs(exp).max())


# revision 42
# speedup vs baseline: 1.1718x; 1.0063x over previous
"""Multi-head cross-attention Trainium2 Bass kernel, SPMD over 8 NeuronCores.

Sharding: core c handles batch b = c//2 and head group g = c%2 (8 of 16 heads).
Each core computes a partial output projection (its heads' W_o rows); the host
sums the two partials per batch element.

Device pipeline per core (all matmuls bf16 with fp32 PSUM accumulation):
  kT = (Wk^T x^T)          [512 hd, 2048 kseq]   (per-partition bias b_k)
  v  = (x Wv)              [2048 kseq, 8*64]
  qT = (Wq^T y^T)          [512 hd, 1024 q]      (per-partition bias b_q)
  per (head-pair, q-tile, k-chunk):
      S^T[k, q|q'] = kT_h^T-chunk @ qT_h for both heads of the pair
        (K=64 row-tiled at partitions 0/64 -> concurrent in the PE array)
      em = exp(0.125 * S^T)  (one ACT op per pair)
      em *= maskT            (one DVE mul per pair, head-broadcast)
      attnV: col-tiled pair into ONE psum bank: h0 -> vacc[0:64],
        h1 -> vacc[64:128]  (tile_position (0,0)/(0,64), concurrent)
      denominators: every 2nd kc, a 4-way col-tiled batch of ones-matmuls
        (strips at psum partitions 0/32/64/96) accumulates sum_k em for
        (even-kc h0, even h1, odd h0, odd h1) -> 2 kc amortized per 512 cyc
  normalize per block: dsum rows -> reciprocal_approx_fast -> bf16 ->
      one K=2 selector matmul (E2) broadcasts [2,512] over the 128
      partitions -> one [128,512] DVE mul into vals
  out_partial = vals^T-chunks @ Wo-rows  -> [1024 q, 1024 D]
Load phase: inputs are host-preswizzled to SBUF layout so each tensor is one
large contiguous DMA (the per-DMA issue cost on the Sync queue is ~0.6us;
many small DMAs made the old load phase issue-bound).  ~30 junk matmuls on a
zeroed tile warm the PE clock (HAM) during the DMA window.
b_v and b_o fold into a host-side constant row (attn rows sum to 1).
"""

import sys
from collections import deque

import numpy as np
import ml_dtypes

if "/opt/trn_rl_repo" not in sys.path:
    sys.path.insert(0, "/opt/trn_rl_repo")

BF = ml_dtypes.bfloat16

B, NKV, NQ, D, H = 4, 2048, 1024, 1024, 16
HD = D // H          # 64
NHL = 8              # heads per core (local)
P = 128
DC = D // P          # 8 contraction chunks over model dim
KC = NKV // P        # 16 key-seq chunks
QT = NQ // 512       # 2 q tiles of 512 for attention
MT = 4               # hd-dim chunks of kT/qT (512/128)

_CACHE = {}


def _build_program():
    import concourse.bass as bass
    import concourse.mybir as mybir
    import concourse.tile as tile
    from concourse import bacc

    f32 = mybir.dt.float32
    bf16 = mybir.dt.bfloat16

    nc = bacc.Bacc(
        "TRN2", target_bir_lowering=False, debug=False, num_devices=8
    )

    # host-preswizzled inputs: each is one contiguous DMA into its SBUF tile
    xT_d = nc.dram_tensor("xT", [512, 4096], bf16, kind="ExternalInput").ap()
    yT_d = nc.dram_tensor("yT", [P, DC * NQ], bf16, kind="ExternalInput").ap()
    maskT_d = nc.dram_tensor(
        "maskT", [P, KC * NQ], bf16, kind="ExternalInput"
    ).ap()
    wk_d = nc.dram_tensor("wk", [P, DC * 512], bf16, kind="ExternalInput").ap()
    wv_d = nc.dram_tensor("wv", [P, DC * 512], bf16, kind="ExternalInput").ap()
    wq_d = nc.dram_tensor("wq", [P, DC * 512], bf16, kind="ExternalInput").ap()
    wo_d = nc.dram_tensor("wo", [P, MT * D], bf16, kind="ExternalInput").ap()
    bk_d = nc.dram_tensor("bk", [P, MT], f32, kind="ExternalInput").ap()
    bq_d = nc.dram_tensor("bq", [P, MT], f32, kind="ExternalInput").ap()
    out_d = nc.dram_tensor("out", [NQ, D], bf16, kind="ExternalOutput").ap()

    Exp = mybir.ActivationFunctionType.Exp

    with tile.TileContext(nc) as tc:
        with (
            tc.tile_pool(name="persist", bufs=1) as persist,
            tc.tile_pool(name="work", bufs=3) as work,
            tc.tile_pool(name="empool", bufs=6) as empool,
            tc.tile_pool(name="pmm", bufs=2, space="PSUM") as pmm,
            tc.tile_pool(name="pacc", bufs=2, space="PSUM") as pacc,
            tc.tile_pool(name="psc", bufs=2, space="PSUM") as psc,
        ):
            # ---- persistent SBUF tiles ----
            wk_sb = persist.tile([P, DC * 512], bf16, tag="wk", name="wk")
            wv_sb = persist.tile([P, DC * 512], bf16, tag="wv", name="wv")
            wq_sb = persist.tile([P, DC * 512], bf16, tag="wq", name="wq")
            wo_sb = persist.tile([P, MT * D], bf16, tag="wo", name="wo")
            xT_sb = persist.tile([P, DC * NKV], bf16, tag="xT", name="xT")
            yT_sb = persist.tile([P, DC * NQ], bf16, tag="yT", name="yT")
            maskT_sb = persist.tile(
                [P, KC * NQ], bf16, tag="mT", name="mT"
            )
            bk_sb = persist.tile([P, MT], f32, tag="bk", name="bk")
            bq_sb = persist.tile([P, MT], f32, tag="bq", name="bq")

            def xs(d, lo, hi):     # xT chunk d, key-cols lo:hi
                return xT_sb[:, d * NKV + lo:d * NKV + hi]

            def ys(d, lo, hi):
                return yT_sb[:, d * NQ + lo:d * NQ + hi]

            # warm tile + constants
            warm = persist.tile([P, 256], bf16, tag="warm", name="warm")
            nc.gpsimd.memset(warm, 0.0)

            ones_row = persist.tile([1, HD], bf16, tag="onesr", name="onesr")
            nc.gpsimd.memset(ones_row, 1.0)

            # ---- HAM warmup: junk matmuls on the zeroed tile ----
            wps = psc.tile([P, 1024], f32, tag="sc", name="warmps")
            for i in range(64):
                nc.tensor.matmul(
                    wps[:, 0:256], lhsT=warm[:, 0:P], rhs=warm,
                    start=True, stop=True,
                )

            # ---- input DMAs: one per tensor (xT by column-quarters) ----
            nc.sync.dma_start(wk_sb, wk_d)
            nc.sync.dma_start(bk_sb, bk_d)
            nc.sync.dma_start(bq_sb, bq_d)
            x3 = xT_sb.rearrange("p (d c) -> p d c", d=DC)

            def load_xq(qb):
                nc.sync.dma_start(
                    x3[:, :, qb * 512:(qb + 1) * 512],
                    xT_d[qb * P:(qb + 1) * P, :].rearrange(
                        "p (d c) -> p d c", d=DC),
                )
            # first quarter in d-halves so the kT chain starts sooner
            nc.sync.dma_start(
                x3[:, 0:4, 0:512],
                xT_d[0:P, 0:4 * 512].rearrange("p (d c) -> p d c", d=4),
            )
            nc.sync.dma_start(
                x3[:, 4:8, 0:512],
                xT_d[0:P, 4 * 512:8 * 512].rearrange("p (d c) -> p d c", d=4),
            )
            nc.sync.dma_start(wv_sb, wv_d)
            for qb in range(1, 4):
                load_xq(qb)
            nc.sync.dma_start(wq_sb, wq_d)
            nc.sync.dma_start(yT_sb, yT_d)
            for mq in range(4):
                nc.sync.dma_start(
                    maskT_sb[:, mq * 4096:(mq + 1) * 4096],
                    maskT_d[:, mq * 4096:(mq + 1) * 4096],
                )
            nc.sync.dma_start(wo_sb, wo_d)

            kT_sb = [
                persist.tile([P, NKV], bf16, tag=f"kT{m}", name=f"kT{m}")
                for m in range(MT)
            ]
            qT_sb = [
                persist.tile([P, NQ], bf16, tag=f"qT{m}", name=f"qT{m}")
                for m in range(MT)
            ]
            v_sb = [
                persist.tile([P, NHL * 65], bf16, tag=f"v{i}", name=f"v{i}")
                for i in range(KC)
            ]
            for i in range(KC):
                nc.gpsimd.memset(
                    v_sb[i].rearrange("p (h c) -> p h c", c=65)[:, :, 64:65],
                    1.0,
                )
            vals_sb = [
                persist.tile([P, NQ], bf16, tag=f"vals{c}", name=f"vals{c}")
                for c in range(MT)
            ]

            # ---- projection op queues (closures; chain tail adds bias) ----
            def proj_ops(m, which, ns=None, pool=None):
                w_sb, dst, bias, ncols = (
                    (wk_sb, kT_sb, bk_sb, NKV) if which == "k"
                    else (wq_sb, qT_sb, bq_sb, NQ)
                )
                src = xs if which == "k" else ys
                pool_ = pool if pool is not None else pmm
                tag = "sc" if pool_ is psc else "mm"
                ops = []
                hold = {}
                nlist = list(range(ncols // 512) if ns is None else ns)
                for n in nlist:
                    for d in range(DC):
                        def op(m=m, n=n, d=d, w_sb=w_sb, dst=dst, bias=bias,
                               which=which, src=src, pool_=pool_, tag=tag):
                            if d == 0:
                                hold[n] = pool_.tile(
                                    [P, 512] if tag == "mm" else [P, 1024],
                                    f32, tag=tag, name=f"pj{which}{m}_{n}"
                                )
                            ps = hold[n][:, 0:512]
                            nc.tensor.matmul(
                                ps,
                                lhsT=w_sb[:, d * 512 + m * P:
                                          d * 512 + (m + 1) * P],
                                rhs=src(d, n * 512, (n + 1) * 512),
                                start=(d == 0),
                                stop=(d == DC - 1),
                            )
                            if d == DC - 1:
                                nc.vector.tensor_scalar_add(
                                    dst[m][:, n * 512:(n + 1) * 512],
                                    ps, bias[:, m:m + 1]
                                )
                        ops.append(op)
                return ops

            def v_ops(i, pool_):
                tag = "sc" if pool_ is psc else "mm"
                ops = []
                hold = {}

                def mk(d, i=i, pool_=pool_, tag=tag):
                    def op(d=d):
                        if d == 0:
                            hold[0] = pool_.tile(
                                [P, 512] if tag == "mm" else [P, 1024],
                                f32, tag=tag, name=f"ps_v{i}"
                            )
                        nc.tensor.matmul(
                            hold[0][:, 0:512],
                            lhsT=xs(d, i * P, (i + 1) * P),
                            rhs=wv_sb[:, d * 512:(d + 1) * 512],
                            start=(d == 0),
                            stop=(d == DC - 1),
                        )
                        if d == DC - 1:
                            v3 = v_sb[i].rearrange("p (h c) -> p h c", c=65)
                            nc.vector.tensor_copy(
                                v3[:, :, 0:64],
                                hold[0][:, 0:512].rearrange(
                                    "p (h c) -> p h c", c=64),
                            )
                    return op
                for d in range(DC):
                    ops.append(mk(d))
                return ops

            # upfront: kT chunk-0 chains as their xT quarters land, early v
            # chains, then qT chunk-0 — attention starts ASAP so ACT/DVE ramp
            for op in proj_ops(0, "k", ns=[0], pool=psc):
                op()
            for i in (0, 1, 2, 3):
                for op in v_ops(i, psc):
                    op()
            for op in proj_ops(0, "k", ns=[1], pool=psc):
                op()
            for i in (4, 5, 6, 7):
                for op in v_ops(i, psc):
                    op()
            for op in proj_ops(0, "k", ns=[2], pool=psc):
                op()
            for i in (8, 9):
                for op in v_ops(i, psc):
                    op()
            for op in proj_ops(0, "k", ns=[3], pool=psc):
                op()
            for i in (10, 11):
                for op in v_ops(i, psc):
                    op()
            for op in proj_ops(0, "q", pool=psc):
                op()

            # deferred work, drained inside the attention loops:
            # hp0 gets the v tail + chunk-1 projections, etc.
            pending = {
                0: deque(
                    v_ops(12, pmm) + v_ops(13, pmm) + v_ops(14, pmm)
                    + v_ops(15, pmm)
                    + proj_ops(1, "k") + proj_ops(1, "q")
                ),
                1: deque(proj_ops(2, "k") + proj_ops(2, "q")),
                2: deque(proj_ops(3, "k") + proj_ops(3, "q")),
                3: deque(),
            }

            # ---- output-projection chains (closures) ----
            def wo_ops(t2s, pool_pick, copy_eng="scalar", dma_eng=None):
                ops = []
                hold = {}
                for t2 in t2s:
                    for n in range(D // 512):
                        for c in range(MT):
                            def op(t2=t2, n=n, c=c):
                                if c == 0:
                                    pool, tag = pool_pick(t2, n)
                                    hold[(t2, n)] = pool.tile(
                                        [P, 512] if tag == "mm"
                                        else [P, 1024],
                                        f32, tag=tag, name=f"ps_o{t2}_{n}"
                                    )
                                ps_o = hold[(t2, n)][:, 0:512]
                                nc.tensor.matmul(
                                    ps_o,
                                    lhsT=vals_sb[c][:, t2 * P:(t2 + 1) * P],
                                    rhs=wo_sb[:, c * D + n * 512:
                                              c * D + (n + 1) * 512],
                                    start=(c == 0),
                                    stop=(c == MT - 1),
                                )
                                if c == MT - 1:
                                    ot = work.tile(
                                        [P, 512], bf16, tag="ot",
                                        name=f"ot{t2}_{n}", bufs=3
                                    )
                                    if copy_eng == "scalar":
                                        nc.scalar.copy(ot, ps_o)
                                    else:
                                        nc.vector.tensor_copy(ot, ps_o)
                                    if dma_eng == "multi":
                                        eng = (nc.sync, nc.scalar,
                                               nc.gpsimd)[(t2 * 2 + n) % 3]
                                    else:
                                        eng = nc.sync
                                    eng.dma_start(
                                        out_d[t2 * P:(t2 + 1) * P,
                                              n * 512:(n + 1) * 512], ot
                                    )
                            ops.append(op)
                return ops

            wo_first = deque(wo_ops(range(0, 4), lambda t2, n: (pmm, "mm")))

            # ---- attention ----
            norm_a = deque()
            norm_b = deque()

            def make_norm(hp, t, ut, sf0, sf1):
                qs = slice(t * 512, (t + 1) * 512)
                rbs = {}

                def mk_na(h, sf):
                    def na():
                        r = work.tile([1, 512], f32, tag=f"r{h}",
                                      name=f"r{h}_{hp}_{t}", bufs=2)
                        nc.vector.reciprocal_approx_fast(r, sf)
                        rb = work.tile([1, 512], bf16, tag=f"rb{h}",
                                       name=f"rb{h}_{hp}_{t}", bufs=2)
                        nc.vector.tensor_copy(rb, r)
                        rbs[h] = rb
                    return na

                def nb():
                    bps = pmm.tile([P, 512], f32, tag="mm",
                                   name=f"bps{hp}_{t}")
                    for a in range(2):
                        nc.tensor.matmul(
                            bps[a * HD:(a + 1) * HD, :],
                            lhsT=ones_row, rhs=rbs[a],
                            start=True, stop=True,
                        )
                    nc.vector.tensor_mul(vals_sb[hp][:, qs], ut, bps)
                return mk_na(0, sf0), mk_na(1, sf1), nb

            # Flat software pipeline over all 8 (hp, t) blocks x 16 kc:
            # iteration g emits scores/exp/mask for global step g and
            # attnV for step g-1 — continuous across block boundaries, so
            # neither ACT nor PE sees a bubble when a block ends.  The
            # epilogue of a finished block rides inside the next block's
            # first iteration (after its mask, so DVE priority holds).
            blocks = [(hp, t) for hp in range(MT) for t in range(QT)]
            NB = len(blocks)
            st = {}      # per-block state: accs, ems
            slots = {hp: 2 * KC for hp in range(MT)}

            sps = {}

            def emit_scores(g):
                bi, kc = divmod(g, KC)
                hp, t = blocks[bi]
                qs = slice(t * 512, (t + 1) * 512)
                if kc == 0:
                    st[bi] = {
                        "accs": [
                            pacc.tile([65, 512], f32, tag="acc",
                                      name=f"acc{bi}_{h}")
                            for h in (0, 1)
                        ],
                        "ems": {},
                    }
                sp2 = psc.tile(
                    [P, 1024], f32, tag="sc", name=f"sp{bi}_{kc}"
                )
                for a in range(2):
                    po = a * HD
                    nc.tensor.matmul(
                        sp2[:, a * 512:(a + 1) * 512],
                        lhsT=kT_sb[hp][po:po + HD, kc * P:(kc + 1) * P],
                        rhs=qT_sb[hp][po:po + HD, qs],
                        start=True,
                        stop=True,
                    )
                sps[g] = sp2

            def emit_expmask(g):
                bi, kc = divmod(g, KC)
                hp, t = blocks[bi]
                sp2 = sps.pop(g)
                em2 = empool.tile(
                    [P, 1024], bf16, tag="em", name=f"em{bi}_{kc}"
                )
                nc.scalar.activation(em2, sp2, Exp, scale=0.125)
                mb = (maskT_sb[:, kc * NQ + t * 512:kc * NQ + (t + 1) * 512]
                      .rearrange("p (o q) -> p o q", o=1)
                      .broadcast_to([P, 2, 512]))
                em3 = em2.rearrange("p (o q) -> p o q", o=2)
                nc.vector.tensor_mul(em3, em3, mb)
                st[bi]["ems"][kc] = em2

            def emit_back(g):
                bj, kd = divmod(g, KC)
                hp, t = blocks[bj]
                s = st[bj]
                emd = s["ems"].pop(kd)
                # attnV: full-array M=65 per head (row 64 = the softmax
                # denominator via the v ones-column)
                for a, h in enumerate((2 * hp, 2 * hp + 1)):
                    nc.tensor.matmul(
                        s["accs"][a],
                        lhsT=v_sb[kd][:, h * 65:(h + 1) * 65],
                        rhs=emd[:, a * 512:(a + 1) * 512],
                        start=(kd == 0),
                        stop=(kd == KC - 1),
                    )
                if kd == KC - 1:
                    # epilogue: drain the acc banks (ut halves on DVE,
                    # denominator rows on ACT), defer recip/cast/bcast
                    ut = work.tile([P, 512], f32, tag="ut",
                                   name=f"ut{bj}", bufs=2)
                    sfs = []
                    for a in range(2):
                        nc.vector.tensor_copy(
                            ut[a * HD:(a + 1) * HD, :],
                            s["accs"][a][0:HD, :])
                        sf = work.tile([1, 512], f32, tag=f"s{a}",
                                       name=f"s{a}_{bj}", bufs=2)
                        nc.scalar.copy(sf, s["accs"][a][64:65, :])
                        sfs.append(sf)
                    na0, na1, nb = make_norm(hp, t, ut, sfs[0], sfs[1])
                    norm_a.append(na0)
                    norm_a.append(na1)
                    norm_b.append(nb)
                    del st[bj]

            NG = NB * KC
            for g in range(NG + 1):
                if g < NG:
                    # scores for g and g+1 adjacent (PE pairs back-to-back
                    # halves the tiled-transition count)
                    if g % 2 == 0:
                        emit_scores(g)
                        emit_scores(g + 1)
                    emit_expmask(g)
                if g >= 1:
                    emit_back(g - 1)
                if g < NG:
                    bi, kc = divmod(g, KC)
                    hp, t = blocks[bi]
                    q = pending[hp]
                    n_emit = -(-len(q) // slots[hp])  # ceil
                    for _ in range(min(n_emit, len(q))):
                        q.popleft()()
                    slots[hp] -= 1
                    last = (bi == NB - 1)
                    if kc == (1 if last else 2) and norm_a:
                        norm_a.popleft()()
                    if kc == (2 if last else 4) and norm_a:
                        norm_a.popleft()()
                    if kc == (3 if last else 6) and norm_b:
                        norm_b.popleft()()
                    if last and kc >= 4:
                        for _ in range(3):
                            if wo_first:
                                wo_first.popleft()()

            while wo_first:
                wo_first.popleft()()

            # ---- output projection, second q-half ----
            ops2 = wo_ops(range(4, NQ // P),
                          lambda t2, n: ((pmm, "mm") if (t2 * 2 + n) % 2 == 0
                                         else (psc, "sc")),
                          copy_eng="vector", dma_eng="multi")
            # sc-tagged chains' first 3 MMs (c=0..2 don't read vals[3] t1)
            # run ahead of the final norm (pmm stays free for its bcast);
            # their c=3 MMs and the remaining chains follow
            for j in (1, 3):
                for c in range(3):
                    ops2[j * MT + c]()
            while norm_a:
                norm_a.popleft()()
            while norm_b:
                norm_b.popleft()()
            for j in (1, 3):
                ops2[j * MT + 3]()
            for j in (0, 2):
                for c in range(MT):
                    ops2[j * MT + c]()
            for i in range(4 * MT, len(ops2)):
                ops2[i]()

    nc.compile()
    return nc


def _get_program():
    if "nc" not in _CACHE:
        _CACHE["nc"] = _build_program()
    return _CACHE["nc"]


def _swizzle_rows(a):
    """[D, C] -> [128, (D//128)*C]: chunk d's rows at free-offset d*C."""
    d128 = a.shape[0] // P
    return np.ascontiguousarray(
        a.reshape(d128, P, -1).transpose(1, 0, 2).reshape(P, -1)
    )


def _per_core_inputs(x, y, mask, W_kv, b_kv, W_q, b_q, W_o):
    """Build the 8 per-core input maps (host-preswizzled to SBUF layouts)."""
    in_maps = []
    mask_f = mask.astype(np.float32)
    for c in range(8):
        b, g = c // 2, c % 2
        gh = np.arange(g * 8, g * 8 + 8)
        k_cols = (gh[:, None] * 2 * HD + np.arange(HD)[None, :]).ravel()
        v_cols = k_cols + HD
        q_cols = slice(g * 512, (g + 1) * 512)

        xT = x[b].T                                  # [D, NKV]
        # xT dram [512, 4096]: row qb*128+p, col d*512+cc =
        #   xT[d*128+p, qb*512+cc]
        xr = xT.reshape(DC, P, 4, 512)               # [d, p, qb, cc]
        x_host = np.ascontiguousarray(
            xr.transpose(2, 1, 0, 3).reshape(512, 4096)).astype(BF)
        yT = y[b].T                                  # [D, NQ]
        y_host = _swizzle_rows(yT).astype(BF)        # [128, 8*1024]
        mT = mask_f[b].T                             # [NKV, NQ]
        m_host = _swizzle_rows(mT).astype(BF)        # [128, 16*1024]
        wk_host = _swizzle_rows(W_kv[:, k_cols]).astype(BF)
        wv_host = _swizzle_rows(W_kv[:, v_cols]).astype(BF)
        wq_host = _swizzle_rows(W_q[:, q_cols]).astype(BF)
        wo_host = _swizzle_rows(W_o[q_cols, :]).astype(BF)
        bk_host = np.ascontiguousarray(
            b_kv[k_cols].astype(np.float32).reshape(MT, P).T)
        bq_host = np.ascontiguousarray(
            b_q[np.arange(g * 512, (g + 1) * 512)]
            .astype(np.float32).reshape(MT, P).T)
        in_maps.append({
            "xT": x_host, "yT": y_host, "maskT": m_host,
            "wk": wk_host, "wv": wv_host, "wq": wq_host, "wo": wo_host,
            "bk": bk_host, "bq": bq_host,
        })
    return in_maps


def kernel(x, y, mask, W_kv, b_kv, W_q, b_q, W_o, b_o):
    from concourse import bass_utils

    x = np.asarray(x, np.float32)
    y = np.asarray(y, np.float32)
    mask = np.asarray(mask)
    W_kv = np.asarray(W_kv, np.float32)
    b_kv = np.asarray(b_kv, np.float32)
    W_q = np.asarray(W_q, np.float32)
    b_q = np.asarray(b_q, np.float32)
    W_o = np.asarray(W_o, np.float32)
    b_o = np.asarray(b_o, np.float32)

    nc = _get_program()
    in_maps = _per_core_inputs(x, y, mask, W_kv, b_kv, W_q, b_q, W_o)
    res = bass_utils.run_bass_kernel_spmd(nc, in_maps, core_ids=list(range(8)))

    # b_v folds into a constant row: attn rows sum to 1, so each head adds
    # b_v_h @ W_o_h to every output row; b_o adds on top.
    v_cols_all = (np.arange(H)[:, None] * 2 * HD + HD
                  + np.arange(HD)[None, :]).ravel()
    const_row = b_kv[v_cols_all].astype(np.float32) @ W_o + b_o

    out = np.empty((B, NQ, D), np.float32)
    for b in range(B):
        out[b] = (res.results[2 * b]["out"].astype(np.float32)
                  + res.results[2 * b + 1]["out"].astype(np.float32)
                  + const_row)
    return out


if __name__ == "__main__":
    import reference

    inputs = {k: np.asarray(v) for k, v in reference.setup_inputs().items()}
    got = kernel(**inputs)
    exp = np.asarray(reference.reference(**inputs))
    err = np.abs(got - exp)
    print("absmax rel err:", err.max() / np.abs(exp).max())
